# revision 31
# baseline (speedup 1.0000x reference)
"""Trainium2 Bass kernel for a 4-layer pre-LN transformer + GEGLU FFN.

Sharding: rows (batch*seq) split across 8 cores; cores 0-3 own batch 0,
cores 4-7 own batch 1 (512 rows each).  Attention needs full-sequence K/V
per batch element, so each 4-core group AllGathers its K/V shards per layer.

Per core, per layer:
  LN (DVE stats + quake-rsqrt), h^T via DMA x-bar transpose, h^T also
  quantized to fp8 (HT8).
  All projections are fp8 DoubleRow matmuls (2 c-chunks contracted per
  instruction); weights are host-prescaled by 64 (sigma=0.02 would be
  denormal in e4m3) and the PSUM-evacuation op folds the 1/64 back in.
  K^T -> bounce -> AllGather (bf16).  V natural fp8 with a ones column per
  head -> AllGather (fp8 bytes in bf16-typed buffers).
  OWN-FIRST attention: scores/exp for the core's own 4 k-tiles (read from
  the pre-collective staging) run while the AllGathers are in flight; the
  3 remote blocks are unloaded with runtime-indexed DMAs (partition_id()
  picks the 3 non-own blocks) and processed after.
  Scores: head pairs in the two 64-row halves of the PE array, with the
  odd head STAGGERED one k-group behind the even head so both heads'
  matmuls are dependency-ready and actually overlap in the array.
  Exp on ACT in 3-PSUM-bank groups, writing fp8 exp-scores.
  AV: fp8 DoubleRow (2 k-tiles per matmul), ones column gives softmax
  sums for free; AV + normalization of the previous pair interleave into
  the score emission.  Normalization: ones-matmul broadcast of the sums
  then reciprocal_approx_fast at base partition 0 (the custom-DVE op
  mishandles nonzero base partitions).
Final LN + GEGLU FFN (fp8 DR matmuls, explicit tanh) + residual -> output.
"""

import numpy as np
import ml_dtypes

B, S, C = 2, 2048, 512
L, H, CH = 4, 8, 64
OD = 4 * CH  # 256
EPS = 1e-5

N_CORES = 8
GROUP = 4          # cores per batch element
ROWS = (B * S) // N_CORES  # 512 rows per core
P = 128
RT = ROWS // P     # 4 row tiles
CCH = C // P       # 4 chunks of the hidden/attention dim
KT = S // P        # 16 k tiles (full sequence)
KTO = ROWS // P    # 4 own k tiles
KTR = KT - KTO     # 12 remote k tiles
VW = H * 80        # fp8 V row layout: 64 data + 1 ones + 15 pad per head
SQRT_K = 0x5F3759DF
WSCALE = 64.0      # host-side fp8 weight prescale

BF16 = ml_dtypes.bfloat16
F8 = ml_dtypes.float8_e4m3fn

_CACHE = {}


def _swz(w, pt):
    # [pt*128, N] -> [128, pt*N] with chunk-major free dim
    n = w.shape[1]
    return np.ascontiguousarray(
        w.reshape(pt, P, n).transpose(1, 0, 2).reshape(P, pt * n)
    )


def _build(flags, n_layers=L, fake_ag=False):
    use_gamma, use_beta, use_bo, use_bg, use_bf = flags
    import concourse.bass as bass
    import concourse.bacc as bacc
    import concourse.mybir as mybir
    import concourse.tile as tile

    dt = mybir.dt
    OP = mybir.AluOpType
    AF = mybir.ActivationFunctionType
    DR = mybir.MatmulPerfMode.DoubleRow
    IWS = 1.0 / WSCALE

    nc = bacc.Bacc("TRN2", target_bir_lowering=False, debug=False,
                   num_devices=N_CORES)
    groups = [list(range(g * GROUP, (g + 1) * GROUP))
              for g in range(N_CORES // GROUP)]

    # ---- DRAM I/O (all weights fp8, prescaled by WSCALE on the host) ----
    x_d = nc.dram_tensor("x", [ROWS, C], dt.float32, kind="ExternalInput")
    wq_d = nc.dram_tensor("wq", [L, P, CCH * C], dt.float8e4, kind="ExternalInput")
    wk_d = nc.dram_tensor("wk", [L, P, CCH * C], dt.float8e4, kind="ExternalInput")
    wv_d = nc.dram_tensor("wv", [L, P, CCH * C], dt.float8e4, kind="ExternalInput")
    wo_d = nc.dram_tensor("wo", [L, P, CCH * C], dt.float8e4, kind="ExternalInput")
    wg_d = nc.dram_tensor("wg", [P, CCH * C], dt.float8e4, kind="ExternalInput")
    wf_d = nc.dram_tensor("wf", [P, 2 * C], dt.float8e4, kind="ExternalInput")
    y_d = nc.dram_tensor("y", [ROWS, C], dt.float32, kind="ExternalOutput")
    if use_gamma:
        gam_d = nc.dram_tensor("gam", [L + 1, P, C], dt.bfloat16, kind="ExternalInput")
    if use_beta:
        bet_d = nc.dram_tensor("bet", [L + 1, P, C], dt.bfloat16, kind="ExternalInput")
    if use_bo:
        bo_d = nc.dram_tensor("bob", [L, P, C], dt.float32, kind="ExternalInput")
    if use_bg:
        bg_d = nc.dram_tensor("bgc", [P, CCH], dt.float32, kind="ExternalInput")
    if use_bf:
        bf_d = nc.dram_tensor("bfb", [P, C], dt.float32, kind="ExternalInput")

    # separate K / V bounce buffers (fp8 bytes in bf16-typed buffers):
    # the small K AllGather launches right after the K projection so it
    # lands before the own-shard score phase runs out of work.
    KHALF = CCH * ROWS // 2
    VHALF = KTO * VW // 2
    kin_k = [nc.dram_tensor(f"kin_k{i}", [P, KHALF], dt.bfloat16,
                            kind="Internal") for i in range(2)]
    kout_k = [nc.dram_tensor(f"kout_k{i}", [GROUP, P, KHALF], dt.bfloat16,
                             kind="Internal") for i in range(2)]
    kin_v = [nc.dram_tensor(f"kin_v{i}", [P, VHALF], dt.bfloat16,
                            kind="Internal") for i in range(2)]
    kout_v = [nc.dram_tensor(f"kout_v{i}", [GROUP, P, VHALF], dt.bfloat16,
                             kind="Internal") for i in range(2)]
    wu_in = nc.dram_tensor("wu_in", [P, 8], dt.bfloat16, kind="Internal")
    wu_out = nc.dram_tensor("wu_out", [GROUP, P, 8], dt.bfloat16, kind="Internal")

    # ---- persistent SBUF ----
    XSB = nc.alloc_sbuf_tensor("xsb", [P, RT * C], dt.float32).ap()
    HSB = nc.alloc_sbuf_tensor("hsb", [P, RT * C], dt.bfloat16).ap()
    HTSB = nc.alloc_sbuf_tensor("htsb", [P, CCH * ROWS], dt.bfloat16).ap()
    HT8 = nc.alloc_sbuf_tensor("ht8", [P, CCH, ROWS], dt.float8e4).ap()
    QTSB = nc.alloc_sbuf_tensor("qtsb", [P, CCH * ROWS], dt.bfloat16).ap()
    KSTG = nc.alloc_sbuf_tensor("kstg", [P, CCH * ROWS], dt.bfloat16).ap()
    K8STG = nc.alloc_sbuf_tensor("k8stg", [P, CCH * ROWS], dt.float8e4).ap()
    KT8 = nc.alloc_sbuf_tensor("kt8", [P, CCH * KTR * P], dt.float8e4).ap()
    KTSB = nc.alloc_sbuf_tensor("ktsb", [P, CCH * KTR * P], dt.bfloat16).ap()
    VSTG = nc.alloc_sbuf_tensor("vstg", [P, KTO, H, 80], dt.float8e4).ap()
    VSB = nc.alloc_sbuf_tensor("vsb", [P, KTR, H, 80], dt.float8e4).ap()
    OSC = nc.alloc_sbuf_tensor("osc", [P, CCH, ROWS], dt.float8e4).ap()
    FFSB = nc.alloc_sbuf_tensor("ffsb", [P, 2, ROWS], dt.float8e4).ap()
    ONES = nc.alloc_sbuf_tensor("ones", [P, P], dt.bfloat16).ap()
    RECB = nc.alloc_sbuf_tensor("recb", [P, 2 * ROWS], dt.bfloat16).ap()

    with tile.TileContext(nc) as tc:
        with (
            tc.tile_pool(name="wpool", bufs=2) as wpool,
            tc.tile_pool(name="epool", bufs=4) as epool,
            tc.tile_pool(name="small", bufs=2) as small,
            tc.tile_pool(name="gpool", bufs=2) as gpool,
            tc.tile_pool(name="mmps", bufs=2, space="PSUM") as mmps,
            tc.tile_pool(name="scps", bufs=2, space="PSUM") as scps,
        ):
            # warmup collective at t=0: overlaps the first AllGather's
            # rendezvous/firmware-init with the x DMA + first LN + K proj
            WUS = small.tile([P, 8], dt.bfloat16, tag="wus", bufs=1)
            nc.vector.memset(WUS[:], 0.0)
            nc.sync.dma_start(wu_in.ap(), WUS[:])
            if not fake_ag:
                nc.gpsimd.collective_compute(
                    "AllGather", mybir.AluOpType.bypass, replica_groups=groups,
                    ins=[wu_in.ap().opt()], outs=[wu_out.ap().opt()])
            # one-time init
            nc.vector.memset(ONES, 1.0)
            nc.vector.memset(VSTG.rearrange("p a b c -> p (a b c)"), 0.0)
            nc.vector.memset(VSTG[:, :, :, 64:65], 1.0)
            nc.sync.dma_start(XSB.rearrange("p (rt c) -> p rt c", rt=RT),
                              x_d.ap().rearrange("(rt p) c -> p rt c", p=P))

            # runtime rank within the replica group (for remote-block DMAs)
            me = nc.sync.partition_id() % GROUP

            def layer_norm(li):
                """x (XSB) -> h bf16 (HSB), h^T (HTSB) + fp8 h^T (HT8)."""
                MV = small.tile([P, 2 * RT], dt.float32, tag="mv")
                for rt in range(RT):
                    st6 = small.tile([P, 6], dt.float32, tag="st6")
                    nc.vector.bn_stats(st6[:], XSB[:, rt * C:(rt + 1) * C])
                    nc.vector.bn_aggr(MV[:, 2 * rt:2 * rt + 2], st6[:])
                var = MV[:].rearrange("p (rt two) -> p two rt", two=2)[:, 1, :]
                VT = small.tile([P, RT], dt.float32, tag="vt")
                VH = small.tile([P, RT], dt.float32, tag="vh")
                KC = small.tile([P, RT], dt.int32, tag="kc")
                R0 = small.tile([P, RT], dt.int32, tag="r0")
                nc.vector.tensor_scalar(VT[:], var, EPS, None, OP.add)
                nc.vector.tensor_scalar(VH[:], VT[:], 0.5, None, OP.mult)
                nc.vector.memset(KC[:], SQRT_K)
                nc.vector.tensor_scalar(R0[:], VT[:].bitcast(dt.int32), 1, None,
                                        OP.logical_shift_right)
                nc.vector.scalar_tensor_tensor(R0[:], KC[:], 0, R0[:],
                                               OP.bypass, OP.subtract)
                r = R0[:].bitcast(dt.float32)
                for _ in range(2):
                    A = small.tile([P, RT], dt.float32, tag="nra")
                    Cc = small.tile([P, RT], dt.float32, tag="nrc")
                    Rn = small.tile([P, RT], dt.float32, tag="nrr")
                    nc.vector.tensor_mul(A[:], r, r)
                    nc.vector.tensor_mul(A[:], A[:], VH[:])
                    nc.vector.tensor_scalar(Cc[:], A[:], -1.0, 1.5, OP.mult, OP.add)
                    nc.vector.tensor_mul(Rn[:], r, Cc[:])
                    r = Rn[:]
                if use_gamma:
                    GT = gpool.tile([P, C], dt.bfloat16, tag="gam")
                    nc.sync.dma_start(GT[:], gam_d.ap()[li])
                if use_beta:
                    BT = gpool.tile([P, C], dt.bfloat16, tag="bet")
                    nc.sync.dma_start(BT[:], bet_d.ap()[li])
                htv = HTSB.rearrange("p (cc r) -> p cc r", cc=CCH)
                for rt in range(RT):
                    dst = HSB[:, rt * C:(rt + 1) * C]
                    nc.vector.tensor_scalar(dst, XSB[:, rt * C:(rt + 1) * C],
                                            MV[:, 2 * rt:2 * rt + 1],
                                            r[:, rt:rt + 1],
                                            OP.subtract, OP.mult)
                    if use_gamma:
                        nc.vector.tensor_mul(dst, dst, GT[:])
                    if use_beta:
                        nc.vector.tensor_add(dst, dst, BT[:])
                    nc.sync.dma_start_transpose(
                        htv[:, :, rt * P:(rt + 1) * P],
                        HSB[:, rt * C:(rt + 1) * C])
                nc.vector.tensor_copy(HT8.rearrange("p a b -> p (a b)"), HTSB)

            def proj_dr(w8, dst_col):
                """fp8 DoubleRow transposed projection -> [c_out, rows]."""
                wv8 = w8.rearrange("p (cc c) -> p cc c", cc=CCH)
                for mc in range(CCH):
                    ps = mmps.tile([P, ROWS], dt.float32, tag="mm")
                    for kp in range(CCH // 2):
                        nc.tensor.matmul(
                            ps[:],
                            lhsT=wv8[:, 2 * kp:2 * kp + 2, mc * P:(mc + 1) * P],
                            rhs=HT8[:, 2 * kp:2 * kp + 2, :],
                            start=(kp == 0), stop=(kp == CCH // 2 - 1),
                            perf_mode=DR)
                    dst_col(mc, ps)

            def attn_layer(li):
                WQ = wpool.tile([P, CCH * C], dt.float8e4, tag="wq")
                WK = wpool.tile([P, CCH * C], dt.float8e4, tag="wk")
                WV = wpool.tile([P, CCH * C], dt.float8e4, tag="wv")
                WO = wpool.tile([P, CCH * C], dt.float8e4, tag="wo")
                nc.sync.dma_start(WK[:], wk_d.ap()[li])
                nc.sync.dma_start(WV[:], wv_d.ap()[li])
                nc.sync.dma_start(WQ[:], wq_d.ap()[li])
                nc.sync.dma_start(WO[:], wo_d.ap()[li])

                layer_norm(li)

                kin_ki, kout_ki = kin_k[li % 2], kout_k[li % 2]
                kin_vi, kout_vi = kin_v[li % 2], kout_v[li % 2]

                # K^T (own shard): bf16 for own scores + fp8 for transport
                def k_out(mc, ps):
                    nc.vector.tensor_scalar(
                        KSTG[:, mc * ROWS:(mc + 1) * ROWS], ps[:], IWS, None,
                        OP.mult)
                    nc.vector.tensor_scalar(
                        K8STG[:, mc * ROWS:(mc + 1) * ROWS], ps[:], IWS, None,
                        OP.mult)
                proj_dr(WK[:], k_out)
                nc.sync.dma_start(kin_ki.ap(), K8STG.bitcast(dt.bfloat16))
                if fake_ag:
                    for r in range(GROUP):
                        nc.sync.dma_start(kout_ki.ap()[r], kin_ki.ap())
                else:
                    nc.gpsimd.collective_compute(
                        "AllGather", mybir.AluOpType.bypass, replica_groups=groups,
                        ins=[kin_ki.ap().opt()], outs=[kout_ki.ap().opt()])

                # V (own shard, natural, fp8, ones col)
                wv8 = WV[:].rearrange("p (cc c) -> p cc c", cc=CCH)
                for kt in range(KTO):
                    ps = mmps.tile([P, C], dt.float32, tag="mm")
                    for kp in range(CCH // 2):
                        nc.tensor.matmul(
                            ps[:],
                            lhsT=HT8[:, 2 * kp:2 * kp + 2, kt * P:(kt + 1) * P],
                            rhs=wv8[:, 2 * kp:2 * kp + 2, :],
                            start=(kp == 0), stop=(kp == CCH // 2 - 1),
                            perf_mode=DR)
                    nc.vector.tensor_scalar(
                        VSTG[:, kt, :, 0:CH],
                        ps[:].rearrange("p (h c) -> p h c", h=H), IWS, None, OP.mult)
                nc.sync.dma_start(
                    kin_vi.ap(),
                    VSTG.rearrange("p a b c -> p (a b c)").bitcast(dt.bfloat16))
                if fake_ag:
                    for r in range(GROUP):
                        nc.sync.dma_start(kout_vi.ap()[r], kin_vi.ap())
                else:
                    nc.gpsimd.collective_compute(
                        "AllGather", mybir.AluOpType.bypass, replica_groups=groups,
                        ins=[kin_vi.ap().opt()], outs=[kout_vi.ap().opt()])

                # Q^T (overlaps the collective)
                proj_dr(WQ[:], lambda mc, ps: nc.vector.tensor_scalar(
                    QTSB[:, mc * ROWS:(mc + 1) * ROWS], ps[:], IWS, None, OP.mult))

                # unload the 3 REMOTE blocks (runtime-indexed by rank)
                if fake_ag:
                    srcs_k = [kout_ki.ap()[j] for j in range(1, GROUP)]
                    srcs_v = [kout_vi.ap()[j] for j in range(1, GROUP)]
                else:
                    srcs_k = [kout_ki.ap()[(me + j) % GROUP] for j in range(1, GROUP)]
                    srcs_v = [kout_vi.ap()[(me + j) % GROUP] for j in range(1, GROUP)]
                kt8v = KT8.rearrange("p (cc k) -> p cc k", cc=CCH)
                ktbv = KTSB.rearrange("p (cc k) -> p cc k", cc=CCH)
                for j in range(1, GROUP):
                    nc.sync.dma_start(
                        kt8v[:, :, (j - 1) * ROWS:j * ROWS].bitcast(dt.bfloat16),
                        srcs_k[j - 1].rearrange("p (cc r) -> p cc r", cc=CCH))
                    # upcast this block to bf16 for the score matmuls
                    nc.vector.tensor_copy(
                        ktbv[:, :, (j - 1) * ROWS:j * ROWS],
                        kt8v[:, :, (j - 1) * ROWS:j * ROWS])
                vsbf = VSB.rearrange("p a b c -> p (a b c)").bitcast(dt.bfloat16)
                for j in range(1, GROUP):
                    nc.sync.dma_start(vsbf[:, (j - 1) * VHALF:j * VHALF],
                                      srcs_v[j - 1])

                # ---- attention ----
                # esb slot s holds k-tile s: slots 0-3 own (KSTG/VSTG),
                # slots 4-15 remote (KTSB/VSB in arrival order).
                OG = [[0, 1, 2], [3]]                 # own k-slot groups
                RG = [[4, 5, 6], [7, 8, 9], [10, 11, 12], [13, 14, 15]]

                def score_mm(hp, sub, ps, j, slot):
                    if slot < KTO:
                        lt = KSTG[sub:sub + CH,
                                  hp * ROWS + slot * P: hp * ROWS + (slot + 1) * P]
                    else:
                        rk = slot - KTO
                        lt = KTSB[sub:sub + CH,
                                  hp * KTR * P + rk * P: hp * KTR * P + (rk + 1) * P]
                    nc.tensor.matmul(
                        ps[:, j * ROWS:(j + 1) * ROWS], lhsT=lt,
                        rhs=QTSB[sub:sub + CH, hp * ROWS:(hp + 1) * ROWS],
                        start=True, stop=True)

                def emit_groups(hp, esA, esB, glist, drain):
                    """staggered A/B score+exp emission over glist."""
                    for gi in range(len(glist) + 1):
                        todo = []
                        if gi >= 1:
                            psB = scps.tile([P, 3 * ROWS], dt.float32, tag="sc")
                            todo.append((CH, psB, esB, glist[gi - 1]))
                        if gi < len(glist):
                            psA = scps.tile([P, 3 * ROWS], dt.float32, tag="sc")
                            todo.append((0, psA, esA, glist[gi]))
                        for j in range(max(len(t[3]) for t in todo)):
                            for sub, ps, es, kts in todo:
                                if j < len(kts):
                                    score_mm(hp, sub, ps, j, kts[j])
                        for sub, ps, es, kts in todo:
                            nc.scalar.activation(
                                es[:, kts[0]:kts[0] + len(kts), :],
                                ps[:, 0:len(kts) * ROWS],
                                AF.Exp, scale=1.0 / np.sqrt(CH))
                        if drain:
                            for _ in range(4):
                                if drain:
                                    drain.pop(0)()

                def make_pair_work(hp, esA, esB):
                    work = []
                    avA = mmps.tile([P, ROWS], dt.float32, tag="mm")
                    avB = mmps.tile([P, ROWS], dt.float32, tag="mm")
                    for t in range(KT // 2):
                        for av, es in ((avA, esA), (avB, esB)):
                            h = 2 * hp + (0 if av is avA else 1)
                            if t < KTO // 2:
                                lt = VSTG[:, 2 * t:2 * t + 2, h, 0:80]
                            else:
                                rt_ = t - KTO // 2
                                lt = VSB[:, 2 * rt_:2 * rt_ + 2, h, 0:80]
                            work.append((lambda av=av, es=es, t=t, lt=lt: nc.tensor.matmul(
                                av[0:80, :], lhsT=lt,
                                rhs=es[:, 2 * t:2 * t + 2, :],
                                start=(t == 0), stop=(t == KT // 2 - 1),
                                perf_mode=DR)))

                    def post_head(j, av):
                        h = 2 * hp + j
                        sub = (h % 2) * CH
                        nc.vector.tensor_copy(
                            OSC[sub:sub + CH, hp, :], av[0:CH, :])
                        nc.vector.tensor_copy(
                            RECB[CH:CH + 1, j * ROWS:(j + 1) * ROWS],
                            av[CH:CH + 1, :])
                        bc = mmps.tile([P, ROWS], dt.float32, tag="mm")
                        nc.tensor.matmul(
                            bc[:, :], lhsT=ONES[CH:CH + 1, :],
                            rhs=RECB[CH:CH + 1, j * ROWS:(j + 1) * ROWS],
                            start=True, stop=True)
                        rb = small.tile([P, ROWS], dt.float32, tag="rb")
                        nc.vector.reciprocal_approx_fast(rb[:], bc[:, :])
                        dst = OSC[sub:sub + CH, hp, :]
                        nc.vector.tensor_mul(dst, dst, rb[sub:sub + CH, :])
                    work.append(lambda: post_head(0, avA))
                    work.append(lambda: post_head(1, avB))
                    return work

                # phase A: own k-tiles for all pairs (overlaps the AllGathers)
                esbs = []
                for hp in range(H // 2):
                    esA = epool.tile([P, KT, ROWS], dt.float8e4, tag="esA")
                    esB = epool.tile([P, KT, ROWS], dt.float8e4, tag="esB")
                    esbs.append((esA, esB))
                    emit_groups(hp, esA, esB, OG, None)
                # phase B: remote k-tiles; AV of the previous pair interleaves
                pending = []
                for hp in range(H // 2):
                    esA, esB = esbs[hp]
                    emit_groups(hp, esA, esB, RG, pending)
                    while pending:
                        pending.pop(0)()
                    pending = make_pair_work(hp, esA, esB)
                while pending:
                    pending.pop(0)()

                # out-projection (fp8 DR) + residual
                if use_bo:
                    BO = gpool.tile([P, C], dt.float32, tag="bo")
                    nc.sync.dma_start(BO[:], bo_d.ap()[li])
                wo8 = WO[:].rearrange("p (cc c) -> p cc c", cc=CCH)
                for rt in range(RT):
                    ps = mmps.tile([P, C], dt.float32, tag="mm")
                    for kp in range(CCH // 2):
                        nc.tensor.matmul(
                            ps[:],
                            lhsT=OSC[:, 2 * kp:2 * kp + 2, rt * P:(rt + 1) * P],
                            rhs=wo8[:, 2 * kp:2 * kp + 2, :],
                            start=(kp == 0), stop=(kp == CCH // 2 - 1),
                            perf_mode=DR)
                    dst = XSB[:, rt * C:(rt + 1) * C]
                    nc.vector.scalar_tensor_tensor(dst, ps[:], IWS, dst,
                                                   OP.mult, OP.add)
                    if use_bo:
                        nc.vector.tensor_add(dst, dst, BO[:])

            for li in range(n_layers):
                attn_layer(li)

            # ---- FFN (fp8 DR) ----
            layer_norm(L)
            WG = wpool.tile([P, CCH * C], dt.float8e4, tag="wq")
            WF = wpool.tile([P, 2 * C], dt.float8e4, tag="wf")
            nc.sync.dma_start(WG[:], wg_d.ap())
            nc.sync.dma_start(WF[:], wf_d.ap())
            if use_bg:
                BG = gpool.tile([P, CCH], dt.float32, tag="bg")
                nc.sync.dma_start(BG[:], bg_d.ap())
            AGT = small.tile([P, 2 * ROWS], dt.bfloat16, tag="ffa")
            GGT = small.tile([P, 2 * ROWS], dt.bfloat16, tag="ffg")
            wg8 = WG[:].rearrange("p (cc c) -> p cc c", cc=CCH)
            for mg in range(CCH):
                ps = mmps.tile([P, ROWS], dt.float32, tag="mm")
                for kp in range(CCH // 2):
                    nc.tensor.matmul(
                        ps[:],
                        lhsT=wg8[:, 2 * kp:2 * kp + 2, mg * P:(mg + 1) * P],
                        rhs=HT8[:, 2 * kp:2 * kp + 2, :],
                        start=(kp == 0), stop=(kp == CCH // 2 - 1),
                        perf_mode=DR)
                dst = (AGT if mg < 2 else GGT)[:, (mg % 2) * ROWS:(mg % 2 + 1) * ROWS]
                if use_bg:
                    nc.vector.tensor_scalar(dst, ps[:], IWS, BG[:, mg:mg + 1],
                                            OP.mult, OP.add)
                else:
                    nc.vector.tensor_scalar(dst, ps[:], IWS, None, OP.mult)
            K1 = 0.7978845608
            for j in range(2):
                ga = GGT[:, j * ROWS:(j + 1) * ROWS]
                aa = AGT[:, j * ROWS:(j + 1) * ROWS]
                SQ = small.tile([P, ROWS], dt.float32, tag="sq")
                WT = small.tile([P, ROWS], dt.float32, tag="wt")
                VV = small.tile([P, ROWS], dt.float32, tag="vv")
                TT = small.tile([P, ROWS], dt.float32, tag="tt")
                HT2 = small.tile([P, ROWS], dt.bfloat16, tag="ht2")
                PP = small.tile([P, ROWS], dt.bfloat16, tag="pp")
                nc.scalar.activation(SQ[:], ga, AF.Square)
                nc.vector.tensor_scalar(WT[:], SQ[:], K1 * 0.044715, K1,
                                        OP.mult, OP.add)
                nc.vector.tensor_mul(VV[:], ga, WT[:])
                nc.scalar.activation(TT[:], VV[:], AF.Tanh)
                nc.vector.tensor_scalar(HT2[:], TT[:], 0.5, 0.5,
                                        OP.mult, OP.add)
                nc.vector.tensor_mul(PP[:], aa, ga)
                nc.vector.tensor_mul(FFSB[:, j, :], PP[:], HT2[:])
            if use_bf:
                BF = gpool.tile([P, C], dt.float32, tag="bf")
                nc.sync.dma_start(BF[:], bf_d.ap())
            wf8 = WF[:].rearrange("p (two c) -> p two c", two=2)
            for rt in range(RT):
                ps = mmps.tile([P, C], dt.float32, tag="mm")
                nc.tensor.matmul(
                    ps[:],
                    lhsT=FFSB[:, 0:2, rt * P:(rt + 1) * P],
                    rhs=wf8[:, 0:2, :],
                    start=True, stop=True, perf_mode=DR)
                OUT = small.tile([P, C], dt.float32, tag="out")
                nc.vector.scalar_tensor_tensor(OUT[:], ps[:], IWS,
                                               XSB[:, rt * C:(rt + 1) * C],
                                               OP.mult, OP.add)
                if use_bf:
                    nc.vector.tensor_add(OUT[:], OUT[:], BF[:])
                nc.sync.dma_start(y_d.ap()[rt * P:(rt + 1) * P, :], OUT[:])

    nc.compile()
    return nc


def kernel(x, ln_gamma, ln_beta, Wq, Wk, Wv, Wo, bo, Wg, bg, Wf, bf):
    x = np.asarray(x, np.float32)
    ln_gamma = np.asarray(ln_gamma, np.float32)
    ln_beta = np.asarray(ln_beta, np.float32)
    Wq, Wk, Wv, Wo = (np.asarray(w, np.float32) for w in (Wq, Wk, Wv, Wo))
    bo = np.asarray(bo, np.float32)
    Wg, Wf = np.asarray(Wg, np.float32), np.asarray(Wf, np.float32)
    bg, bf = np.asarray(bg, np.float32), np.asarray(bf, np.float32)

    use_gamma = not np.all(ln_gamma == 1.0)
    use_beta = not np.all(ln_beta == 0.0)
    use_bo = not np.all(bo == 0.0)
    use_bg = not np.all(bg == 0.0)
    use_bf = not np.all(bf == 0.0)
    flags = (use_gamma, use_beta, use_bo, use_bg, use_bf)

    if flags not in _CACHE:
        _CACHE[flags] = _build(flags)
    nc = _CACHE[flags]

    ws = WSCALE
    wq_h = np.stack([_swz(Wq[l] * ws, CCH) for l in range(L)]).astype(F8)
    wk_h = np.stack([_swz(Wk[l] * ws, CCH) for l in range(L)]).astype(F8)
    wv_h = np.stack([_swz(Wv[l] * ws, CCH) for l in range(L)]).astype(F8)
    wo_h = np.stack([_swz(Wo[l] * ws, CCH) for l in range(L)]).astype(F8)
    wg_h = _swz(Wg * ws, CCH).astype(F8)
    wf_h = _swz(Wf * ws, 2).astype(F8)

    xf = x.reshape(B * S, C)
    base = {
        "wq": wq_h, "wk": wk_h, "wv": wv_h, "wo": wo_h,
        "wg": wg_h, "wf": wf_h,
    }
    if use_gamma:
        base["gam"] = np.ascontiguousarray(
            np.broadcast_to(ln_gamma[:, None, :], (L + 1, P, C))).astype(BF16)
    if use_beta:
        base["bet"] = np.ascontiguousarray(
            np.broadcast_to(ln_beta[:, None, :], (L + 1, P, C))).astype(BF16)
    if use_bo:
        base["bob"] = np.ascontiguousarray(
            np.broadcast_to(bo[:, None, :], (L, P, C))).astype(np.float32)
    if use_bg:
        base["bgc"] = np.ascontiguousarray(bg.reshape(CCH, P).T).astype(np.float32)
    if use_bf:
        base["bfb"] = np.ascontiguousarray(
            np.broadcast_to(bf[None, :], (P, C))).astype(np.float32)

    in_maps = []
    for c in range(N_CORES):
        m = dict(base)
        m["x"] = np.ascontiguousarray(xf[c * ROWS:(c + 1) * ROWS])
        in_maps.append(m)

    from concourse.bass_utils import run_bass_kernel_spmd
    res = run_bass_kernel_spmd(nc, in_maps, core_ids=list(range(N_CORES)))
    out = np.concatenate([res.results[c]["y"] for c in range(N_CORES)], axis=0)
    return out.reshape(B, S, C).astype(np.float32)


# revision 32
# speedup vs baseline: 1.0477x; 1.0477x over previous
"""Trainium2 Bass kernel for a 4-layer pre-LN transformer + GEGLU FFN.

Sharding: rows (batch*seq) split across 8 cores; cores 0-3 own batch 0,
cores 4-7 own batch 1 (512 rows each).  Attention needs full-sequence K/V
per batch element, so each 4-core group AllGathers its K/V shards per layer.

Per core, per layer:
  LN (DVE stats + quake-rsqrt), h^T via DMA x-bar transpose, h^T also
  quantized to fp8 (HT8).
  All projections are fp8 DoubleRow matmuls (2 c-chunks contracted per
  instruction); weights are host-prescaled by 64 (sigma=0.02 would be
  denormal in e4m3) and the PSUM-evacuation op folds the 1/64 back in.
  K^T -> bounce -> AllGather (bf16).  V natural fp8 with a ones column per
  head -> AllGather (fp8 bytes in bf16-typed buffers).
  OWN-FIRST attention: scores/exp for the core's own 4 k-tiles (read from
  the pre-collective staging) run while the AllGathers are in flight; the
  3 remote blocks are unloaded with runtime-indexed DMAs (partition_id()
  picks the 3 non-own blocks) and processed after.
  Scores: head pairs in the two 64-row halves of the PE array, with the
  odd head STAGGERED one k-group behind the even head so both heads'
  matmuls are dependency-ready and actually overlap in the array.
  Exp on ACT in 3-PSUM-bank groups, writing fp8 exp-scores.
  AV: fp8 DoubleRow (2 k-tiles per matmul), ones column gives softmax
  sums for free; AV + normalization of the previous pair interleave into
  the score emission.  Normalization: ones-matmul broadcast of the sums
  then reciprocal_approx_fast at base partition 0 (the custom-DVE op
  mishandles nonzero base partitions).
Final LN + GEGLU FFN (fp8 DR matmuls, explicit tanh) + residual -> output.
"""

import numpy as np
import ml_dtypes

B, S, C = 2, 2048, 512
L, H, CH = 4, 8, 64
OD = 4 * CH  # 256
EPS = 1e-5

N_CORES = 8
GROUP = 4          # cores per batch element
ROWS = (B * S) // N_CORES  # 512 rows per core
P = 128
RT = ROWS // P     # 4 row tiles
CCH = C // P       # 4 chunks of the hidden/attention dim
KT = S // P        # 16 k tiles (full sequence)
KTO = ROWS // P    # 4 own k tiles
KTR = KT - KTO     # 12 remote k tiles
VW = H * 80        # fp8 V row layout: 64 data + 1 ones + 15 pad per head
SQRT_K = 0x5F3759DF
WSCALE = 64.0      # host-side fp8 weight prescale

BF16 = ml_dtypes.bfloat16
F8 = ml_dtypes.float8_e4m3fn

_CACHE = {}


def _swz(w, pt):
    # [pt*128, N] -> [128, pt*N] with chunk-major free dim
    n = w.shape[1]
    return np.ascontiguousarray(
        w.reshape(pt, P, n).transpose(1, 0, 2).reshape(P, pt * n)
    )


def _build(flags, n_layers=L, fake_ag=False):
    use_gamma, use_beta, use_bo, use_bg, use_bf = flags
    import concourse.bass as bass
    import concourse.bacc as bacc
    import concourse.mybir as mybir
    import concourse.tile as tile

    dt = mybir.dt
    OP = mybir.AluOpType
    AF = mybir.ActivationFunctionType
    DR = mybir.MatmulPerfMode.DoubleRow
    IWS = 1.0 / WSCALE

    nc = bacc.Bacc("TRN2", target_bir_lowering=False, debug=False,
                   num_devices=N_CORES)
    groups = [list(range(g * GROUP, (g + 1) * GROUP))
              for g in range(N_CORES // GROUP)]

    # ---- DRAM I/O (all weights fp8, prescaled by WSCALE on the host) ----
    x_d = nc.dram_tensor("x", [ROWS, C], dt.float32, kind="ExternalInput")
    wq_d = nc.dram_tensor("wq", [L, P, CCH * C], dt.float8e4, kind="ExternalInput")
    wk_d = nc.dram_tensor("wk", [L, P, CCH * C], dt.float8e4, kind="ExternalInput")
    wv_d = nc.dram_tensor("wv", [L, P, CCH * C], dt.float8e4, kind="ExternalInput")
    wo_d = nc.dram_tensor("wo", [L, P, CCH * C], dt.float8e4, kind="ExternalInput")
    wg_d = nc.dram_tensor("wg", [P, CCH * C], dt.float8e4, kind="ExternalInput")
    wf_d = nc.dram_tensor("wf", [P, 2 * C], dt.float8e4, kind="ExternalInput")
    y_d = nc.dram_tensor("y", [ROWS, C], dt.float32, kind="ExternalOutput")
    if use_gamma:
        gam_d = nc.dram_tensor("gam", [L + 1, P, C], dt.bfloat16, kind="ExternalInput")
    if use_beta:
        bet_d = nc.dram_tensor("bet", [L + 1, P, C], dt.bfloat16, kind="ExternalInput")
    if use_bo:
        bo_d = nc.dram_tensor("bob", [L, P, C], dt.float32, kind="ExternalInput")
    if use_bg:
        bg_d = nc.dram_tensor("bgc", [P, CCH], dt.float32, kind="ExternalInput")
    if use_bf:
        bf_d = nc.dram_tensor("bfb", [P, C], dt.float32, kind="ExternalInput")

    # separate K / V bounce buffers (fp8 bytes in bf16-typed buffers):
    # the small K AllGather launches right after the K projection so it
    # lands before the own-shard score phase runs out of work.
    KHALF = CCH * ROWS // 2
    VHALF = KTO * VW // 2
    kin_k = [nc.dram_tensor(f"kin_k{i}", [P, KHALF], dt.bfloat16,
                            kind="Internal") for i in range(2)]
    kout_k = [nc.dram_tensor(f"kout_k{i}", [GROUP, P, KHALF], dt.bfloat16,
                             kind="Internal") for i in range(2)]
    kin_v = [nc.dram_tensor(f"kin_v{i}", [P, VHALF], dt.bfloat16,
                            kind="Internal") for i in range(2)]
    kout_v = [nc.dram_tensor(f"kout_v{i}", [GROUP, P, VHALF], dt.bfloat16,
                             kind="Internal") for i in range(2)]

    # ---- persistent SBUF ----
    XSB = nc.alloc_sbuf_tensor("xsb", [P, RT * C], dt.float32).ap()
    HSB = nc.alloc_sbuf_tensor("hsb", [P, RT * C], dt.bfloat16).ap()
    HTSB = nc.alloc_sbuf_tensor("htsb", [P, CCH * ROWS], dt.bfloat16).ap()
    HT8 = nc.alloc_sbuf_tensor("ht8", [P, CCH, ROWS], dt.float8e4).ap()
    QTSB = nc.alloc_sbuf_tensor("qtsb", [P, CCH * ROWS], dt.bfloat16).ap()
    KSTG = nc.alloc_sbuf_tensor("kstg", [P, CCH * ROWS], dt.bfloat16).ap()
    K8STG = nc.alloc_sbuf_tensor("k8stg", [P, CCH * ROWS], dt.float8e4).ap()
    KT8 = nc.alloc_sbuf_tensor("kt8", [P, CCH * KTR * P], dt.float8e4).ap()
    KTSB = nc.alloc_sbuf_tensor("ktsb", [P, CCH * KTR * P], dt.bfloat16).ap()
    VSTG = nc.alloc_sbuf_tensor("vstg", [P, KTO, H, 80], dt.float8e4).ap()
    VSB = nc.alloc_sbuf_tensor("vsb", [P, KTR, H, 80], dt.float8e4).ap()
    OSC = nc.alloc_sbuf_tensor("osc", [P, CCH, ROWS], dt.float8e4).ap()
    FFSB = nc.alloc_sbuf_tensor("ffsb", [P, 2, ROWS], dt.float8e4).ap()
    ONES = nc.alloc_sbuf_tensor("ones", [P, P], dt.bfloat16).ap()
    RECB = nc.alloc_sbuf_tensor("recb", [P, 2 * ROWS], dt.bfloat16).ap()

    with tile.TileContext(nc) as tc:
        with (
            tc.tile_pool(name="wpool", bufs=2) as wpool,
            tc.tile_pool(name="epool", bufs=4) as epool,
            tc.tile_pool(name="small", bufs=2) as small,
            tc.tile_pool(name="gpool", bufs=2) as gpool,
            tc.tile_pool(name="mmps", bufs=2, space="PSUM") as mmps,
            tc.tile_pool(name="scps", bufs=2, space="PSUM") as scps,
        ):
            # one-time init
            nc.vector.memset(ONES, 1.0)
            nc.vector.memset(VSTG.rearrange("p a b c -> p (a b c)"), 0.0)
            nc.vector.memset(VSTG[:, :, :, 64:65], 1.0)
            nc.sync.dma_start(XSB.rearrange("p (rt c) -> p rt c", rt=RT),
                              x_d.ap().rearrange("(rt p) c -> p rt c", p=P))

            # runtime rank within the replica group (for remote-block DMAs)
            me = nc.sync.partition_id() % GROUP

            def layer_norm(li):
                """x (XSB) -> h bf16 (HSB), h^T (HTSB) + fp8 h^T (HT8)."""
                MV = small.tile([P, 2 * RT], dt.float32, tag="mv")
                for rt in range(RT):
                    st6 = small.tile([P, 6], dt.float32, tag="st6")
                    nc.vector.bn_stats(st6[:], XSB[:, rt * C:(rt + 1) * C])
                    nc.vector.bn_aggr(MV[:, 2 * rt:2 * rt + 2], st6[:])
                var = MV[:].rearrange("p (rt two) -> p two rt", two=2)[:, 1, :]
                VT = small.tile([P, RT], dt.float32, tag="vt")
                VH = small.tile([P, RT], dt.float32, tag="vh")
                KC = small.tile([P, RT], dt.int32, tag="kc")
                R0 = small.tile([P, RT], dt.int32, tag="r0")
                nc.vector.tensor_scalar(VT[:], var, EPS, None, OP.add)
                nc.vector.tensor_scalar(VH[:], VT[:], 0.5, None, OP.mult)
                nc.vector.memset(KC[:], SQRT_K)
                nc.vector.tensor_scalar(R0[:], VT[:].bitcast(dt.int32), 1, None,
                                        OP.logical_shift_right)
                nc.vector.scalar_tensor_tensor(R0[:], KC[:], 0, R0[:],
                                               OP.bypass, OP.subtract)
                r = R0[:].bitcast(dt.float32)
                for _ in range(2):
                    A = small.tile([P, RT], dt.float32, tag="nra")
                    Cc = small.tile([P, RT], dt.float32, tag="nrc")
                    Rn = small.tile([P, RT], dt.float32, tag="nrr")
                    nc.vector.tensor_mul(A[:], r, r)
                    nc.vector.tensor_mul(A[:], A[:], VH[:])
                    nc.vector.tensor_scalar(Cc[:], A[:], -1.0, 1.5, OP.mult, OP.add)
                    nc.vector.tensor_mul(Rn[:], r, Cc[:])
                    r = Rn[:]
                if use_gamma:
                    GT = gpool.tile([P, C], dt.bfloat16, tag="gam")
                    nc.sync.dma_start(GT[:], gam_d.ap()[li])
                if use_beta:
                    BT = gpool.tile([P, C], dt.bfloat16, tag="bet")
                    nc.sync.dma_start(BT[:], bet_d.ap()[li])
                htv = HTSB.rearrange("p (cc r) -> p cc r", cc=CCH)
                for rt in range(RT):
                    dst = HSB[:, rt * C:(rt + 1) * C]
                    nc.vector.tensor_scalar(dst, XSB[:, rt * C:(rt + 1) * C],
                                            MV[:, 2 * rt:2 * rt + 1],
                                            r[:, rt:rt + 1],
                                            OP.subtract, OP.mult)
                    if use_gamma:
                        nc.vector.tensor_mul(dst, dst, GT[:])
                    if use_beta:
                        nc.vector.tensor_add(dst, dst, BT[:])
                    nc.sync.dma_start_transpose(
                        htv[:, :, rt * P:(rt + 1) * P],
                        HSB[:, rt * C:(rt + 1) * C])
                nc.vector.tensor_copy(HT8.rearrange("p a b -> p (a b)"), HTSB)

            def proj_dr(w8, dst_col):
                """fp8 DoubleRow transposed projection -> [c_out, rows]."""
                wv8 = w8.rearrange("p (cc c) -> p cc c", cc=CCH)
                for mc in range(CCH):
                    ps = mmps.tile([P, ROWS], dt.float32, tag="mm")
                    for kp in range(CCH // 2):
                        nc.tensor.matmul(
                            ps[:],
                            lhsT=wv8[:, 2 * kp:2 * kp + 2, mc * P:(mc + 1) * P],
                            rhs=HT8[:, 2 * kp:2 * kp + 2, :],
                            start=(kp == 0), stop=(kp == CCH // 2 - 1),
                            perf_mode=DR)
                    dst_col(mc, ps)

            def attn_layer(li):
                WQ = wpool.tile([P, CCH * C], dt.float8e4, tag="wq")
                WK = wpool.tile([P, CCH * C], dt.float8e4, tag="wk")
                WV = wpool.tile([P, CCH * C], dt.float8e4, tag="wv")
                WO = wpool.tile([P, CCH * C], dt.float8e4, tag="wo")
                nc.sync.dma_start(WK[:], wk_d.ap()[li])
                nc.sync.dma_start(WV[:], wv_d.ap()[li])
                nc.sync.dma_start(WQ[:], wq_d.ap()[li])
                nc.sync.dma_start(WO[:], wo_d.ap()[li])

                layer_norm(li)

                kin_ki, kout_ki = kin_k[li % 2], kout_k[li % 2]
                kin_vi, kout_vi = kin_v[li % 2], kout_v[li % 2]

                # K^T (own shard): bf16 for own scores + fp8 for transport
                def k_out(mc, ps):
                    nc.vector.tensor_scalar(
                        KSTG[:, mc * ROWS:(mc + 1) * ROWS], ps[:], IWS, None,
                        OP.mult)
                    nc.vector.tensor_scalar(
                        K8STG[:, mc * ROWS:(mc + 1) * ROWS], ps[:], IWS, None,
                        OP.mult)
                proj_dr(WK[:], k_out)
                nc.sync.dma_start(kin_ki.ap(), K8STG.bitcast(dt.bfloat16))
                if fake_ag:
                    for r in range(GROUP):
                        nc.sync.dma_start(kout_ki.ap()[r], kin_ki.ap())
                else:
                    nc.gpsimd.collective_compute(
                        "AllGather", mybir.AluOpType.bypass, replica_groups=groups,
                        ins=[kin_ki.ap().opt()], outs=[kout_ki.ap().opt()])

                # V (own shard, natural, fp8, ones col)
                wv8 = WV[:].rearrange("p (cc c) -> p cc c", cc=CCH)
                for kt in range(KTO):
                    ps = mmps.tile([P, C], dt.float32, tag="mm")
                    for kp in range(CCH // 2):
                        nc.tensor.matmul(
                            ps[:],
                            lhsT=HT8[:, 2 * kp:2 * kp + 2, kt * P:(kt + 1) * P],
                            rhs=wv8[:, 2 * kp:2 * kp + 2, :],
                            start=(kp == 0), stop=(kp == CCH // 2 - 1),
                            perf_mode=DR)
                    nc.vector.tensor_scalar(
                        VSTG[:, kt, :, 0:CH],
                        ps[:].rearrange("p (h c) -> p h c", h=H), IWS, None, OP.mult)
                nc.sync.dma_start(
                    kin_vi.ap(),
                    VSTG.rearrange("p a b c -> p (a b c)").bitcast(dt.bfloat16))
                if fake_ag:
                    for r in range(GROUP):
                        nc.sync.dma_start(kout_vi.ap()[r], kin_vi.ap())
                else:
                    nc.gpsimd.collective_compute(
                        "AllGather", mybir.AluOpType.bypass, replica_groups=groups,
                        ins=[kin_vi.ap().opt()], outs=[kout_vi.ap().opt()])

                # Q^T (overlaps the collective)
                proj_dr(WQ[:], lambda mc, ps: nc.vector.tensor_scalar(
                    QTSB[:, mc * ROWS:(mc + 1) * ROWS], ps[:], IWS, None, OP.mult))

                # unload the 3 REMOTE blocks (runtime-indexed by rank)
                if fake_ag:
                    srcs_k = [kout_ki.ap()[j] for j in range(1, GROUP)]
                    srcs_v = [kout_vi.ap()[j] for j in range(1, GROUP)]
                else:
                    srcs_k = [kout_ki.ap()[(me + j) % GROUP] for j in range(1, GROUP)]
                    srcs_v = [kout_vi.ap()[(me + j) % GROUP] for j in range(1, GROUP)]
                kt8v = KT8.rearrange("p (cc k) -> p cc k", cc=CCH)
                ktbv = KTSB.rearrange("p (cc k) -> p cc k", cc=CCH)
                for j in range(1, GROUP):
                    nc.sync.dma_start(
                        kt8v[:, :, (j - 1) * ROWS:j * ROWS].bitcast(dt.bfloat16),
                        srcs_k[j - 1].rearrange("p (cc r) -> p cc r", cc=CCH))
                    # upcast this block to bf16 for the score matmuls
                    nc.vector.tensor_copy(
                        ktbv[:, :, (j - 1) * ROWS:j * ROWS],
                        kt8v[:, :, (j - 1) * ROWS:j * ROWS])
                vsbf = VSB.rearrange("p a b c -> p (a b c)").bitcast(dt.bfloat16)
                for j in range(1, GROUP):
                    nc.sync.dma_start(vsbf[:, (j - 1) * VHALF:j * VHALF],
                                      srcs_v[j - 1])

                # ---- attention ----
                # esb slot s holds k-tile s: slots 0-3 own (KSTG/VSTG),
                # slots 4-15 remote (KTSB/VSB in arrival order).
                OG = [[0, 1, 2], [3]]                 # own k-slot groups
                RG = [[4, 5, 6], [7, 8, 9], [10, 11, 12], [13, 14, 15]]

                def score_mm(hp, sub, ps, j, slot):
                    if slot < KTO:
                        lt = KSTG[sub:sub + CH,
                                  hp * ROWS + slot * P: hp * ROWS + (slot + 1) * P]
                    else:
                        rk = slot - KTO
                        lt = KTSB[sub:sub + CH,
                                  hp * KTR * P + rk * P: hp * KTR * P + (rk + 1) * P]
                    nc.tensor.matmul(
                        ps[:, j * ROWS:(j + 1) * ROWS], lhsT=lt,
                        rhs=QTSB[sub:sub + CH, hp * ROWS:(hp + 1) * ROWS],
                        start=True, stop=True)

                def emit_groups(hp, esA, esB, glist, drain):
                    """staggered A/B score+exp emission over glist."""
                    for gi in range(len(glist) + 1):
                        todo = []
                        if gi >= 1:
                            psB = scps.tile([P, 3 * ROWS], dt.float32, tag="sc")
                            todo.append((CH, psB, esB, glist[gi - 1]))
                        if gi < len(glist):
                            psA = scps.tile([P, 3 * ROWS], dt.float32, tag="sc")
                            todo.append((0, psA, esA, glist[gi]))
                        for j in range(max(len(t[3]) for t in todo)):
                            for sub, ps, es, kts in todo:
                                if j < len(kts):
                                    score_mm(hp, sub, ps, j, kts[j])
                        for sub, ps, es, kts in todo:
                            nc.scalar.activation(
                                es[:, kts[0]:kts[0] + len(kts), :],
                                ps[:, 0:len(kts) * ROWS],
                                AF.Exp, scale=1.0 / np.sqrt(CH))
                        if drain:
                            for _ in range(4):
                                if drain:
                                    drain.pop(0)()

                def make_pair_work(hp, esA, esB):
                    work = []
                    avA = mmps.tile([P, ROWS], dt.float32, tag="mm")
                    avB = mmps.tile([P, ROWS], dt.float32, tag="mm")
                    for t in range(KT // 2):
                        for av, es in ((avA, esA), (avB, esB)):
                            h = 2 * hp + (0 if av is avA else 1)
                            if t < KTO // 2:
                                lt = VSTG[:, 2 * t:2 * t + 2, h, 0:80]
                            else:
                                rt_ = t - KTO // 2
                                lt = VSB[:, 2 * rt_:2 * rt_ + 2, h, 0:80]
                            work.append((lambda av=av, es=es, t=t, lt=lt: nc.tensor.matmul(
                                av[0:80, :], lhsT=lt,
                                rhs=es[:, 2 * t:2 * t + 2, :],
                                start=(t == 0), stop=(t == KT // 2 - 1),
                                perf_mode=DR)))

                    def post_head(j, av):
                        h = 2 * hp + j
                        sub = (h % 2) * CH
                        nc.vector.tensor_copy(
                            OSC[sub:sub + CH, hp, :], av[0:CH, :])
                        nc.vector.tensor_copy(
                            RECB[CH:CH + 1, j * ROWS:(j + 1) * ROWS],
                            av[CH:CH + 1, :])
                        bc = mmps.tile([P, ROWS], dt.float32, tag="mm")
                        nc.tensor.matmul(
                            bc[:, :], lhsT=ONES[CH:CH + 1, :],
                            rhs=RECB[CH:CH + 1, j * ROWS:(j + 1) * ROWS],
                            start=True, stop=True)
                        rb = small.tile([P, ROWS], dt.float32, tag="rb")
                        nc.vector.reciprocal_approx_fast(rb[:], bc[:, :])
                        dst = OSC[sub:sub + CH, hp, :]
                        nc.vector.tensor_mul(dst, dst, rb[sub:sub + CH, :])
                    work.append(lambda: post_head(0, avA))
                    work.append(lambda: post_head(1, avB))
                    return work

                # phase A: own k-tiles for all pairs (overlaps the AllGathers)
                esbs = []
                for hp in range(H // 2):
                    esA = epool.tile([P, KT, ROWS], dt.float8e4, tag="esA")
                    esB = epool.tile([P, KT, ROWS], dt.float8e4, tag="esB")
                    esbs.append((esA, esB))
                    emit_groups(hp, esA, esB, OG, None)
                # phase B: remote k-tiles; AV of the previous pair interleaves
                pending = []
                for hp in range(H // 2):
                    esA, esB = esbs[hp]
                    emit_groups(hp, esA, esB, RG, pending)
                    while pending:
                        pending.pop(0)()
                    pending = make_pair_work(hp, esA, esB)
                while pending:
                    pending.pop(0)()

                # out-projection (fp8 DR) + residual
                if use_bo:
                    BO = gpool.tile([P, C], dt.float32, tag="bo")
                    nc.sync.dma_start(BO[:], bo_d.ap()[li])
                wo8 = WO[:].rearrange("p (cc c) -> p cc c", cc=CCH)
                for rt in range(RT):
                    ps = mmps.tile([P, C], dt.float32, tag="mm")
                    for kp in range(CCH // 2):
                        nc.tensor.matmul(
                            ps[:],
                            lhsT=OSC[:, 2 * kp:2 * kp + 2, rt * P:(rt + 1) * P],
                            rhs=wo8[:, 2 * kp:2 * kp + 2, :],
                            start=(kp == 0), stop=(kp == CCH // 2 - 1),
                            perf_mode=DR)
                    dst = XSB[:, rt * C:(rt + 1) * C]
                    nc.vector.scalar_tensor_tensor(dst, ps[:], IWS, dst,
                                                   OP.mult, OP.add)
                    if use_bo:
                        nc.vector.tensor_add(dst, dst, BO[:])

            for li in range(n_layers):
                attn_layer(li)

            # ---- FFN (fp8 DR) ----
            layer_norm(L)
            WG = wpool.tile([P, CCH * C], dt.float8e4, tag="wq")
            WF = wpool.tile([P, 2 * C], dt.float8e4, tag="wf")
            nc.sync.dma_start(WG[:], wg_d.ap())
            nc.sync.dma_start(WF[:], wf_d.ap())
            if use_bg:
                BG = gpool.tile([P, CCH], dt.float32, tag="bg")
                nc.sync.dma_start(BG[:], bg_d.ap())
            AGT = small.tile([P, 2 * ROWS], dt.bfloat16, tag="ffa")
            GGT = small.tile([P, 2 * ROWS], dt.bfloat16, tag="ffg")
            wg8 = WG[:].rearrange("p (cc c) -> p cc c", cc=CCH)
            for mg in range(CCH):
                ps = mmps.tile([P, ROWS], dt.float32, tag="mm")
                for kp in range(CCH // 2):
                    nc.tensor.matmul(
                        ps[:],
                        lhsT=wg8[:, 2 * kp:2 * kp + 2, mg * P:(mg + 1) * P],
                        rhs=HT8[:, 2 * kp:2 * kp + 2, :],
                        start=(kp == 0), stop=(kp == CCH // 2 - 1),
                        perf_mode=DR)
                dst = (AGT if mg < 2 else GGT)[:, (mg % 2) * ROWS:(mg % 2 + 1) * ROWS]
                if use_bg:
                    nc.vector.tensor_scalar(dst, ps[:], IWS, BG[:, mg:mg + 1],
                                            OP.mult, OP.add)
                else:
                    nc.vector.tensor_scalar(dst, ps[:], IWS, None, OP.mult)
            K1 = 0.7978845608
            for j in range(2):
                ga = GGT[:, j * ROWS:(j + 1) * ROWS]
                aa = AGT[:, j * ROWS:(j + 1) * ROWS]
                SQ = small.tile([P, ROWS], dt.float32, tag="sq")
                WT = small.tile([P, ROWS], dt.float32, tag="wt")
                VV = small.tile([P, ROWS], dt.float32, tag="vv")
                TT = small.tile([P, ROWS], dt.float32, tag="tt")
                HT2 = small.tile([P, ROWS], dt.bfloat16, tag="ht2")
                PP = small.tile([P, ROWS], dt.bfloat16, tag="pp")
                nc.scalar.activation(SQ[:], ga, AF.Square)
                nc.vector.tensor_scalar(WT[:], SQ[:], K1 * 0.044715, K1,
                                        OP.mult, OP.add)
                nc.vector.tensor_mul(VV[:], ga, WT[:])
                nc.scalar.activation(TT[:], VV[:], AF.Tanh)
                nc.vector.tensor_scalar(HT2[:], TT[:], 0.5, 0.5,
                                        OP.mult, OP.add)
                nc.vector.tensor_mul(PP[:], aa, ga)
                nc.vector.tensor_mul(FFSB[:, j, :], PP[:], HT2[:])
            if use_bf:
                BF = gpool.tile([P, C], dt.float32, tag="bf")
                nc.sync.dma_start(BF[:], bf_d.ap())
            wf8 = WF[:].rearrange("p (two c) -> p two c", two=2)
            for rt in range(RT):
                ps = mmps.tile([P, C], dt.float32, tag="mm")
                nc.tensor.matmul(
                    ps[:],
                    lhsT=FFSB[:, 0:2, rt * P:(rt + 1) * P],
                    rhs=wf8[:, 0:2, :],
                    start=True, stop=True, perf_mode=DR)
                OUT = small.tile([P, C], dt.float32, tag="out")
                nc.vector.scalar_tensor_tensor(OUT[:], ps[:], IWS,
                                               XSB[:, rt * C:(rt + 1) * C],
                                               OP.mult, OP.add)
                if use_bf:
                    nc.vector.tensor_add(OUT[:], OUT[:], BF[:])
                nc.sync.dma_start(y_d.ap()[rt * P:(rt + 1) * P, :], OUT[:])

    nc.compile()
    return nc


def kernel(x, ln_gamma, ln_beta, Wq, Wk, Wv, Wo, bo, Wg, bg, Wf, bf):
    x = np.asarray(x, np.float32)
    ln_gamma = np.asarray(ln_gamma, np.float32)
    ln_beta = np.asarray(ln_beta, np.float32)
    Wq, Wk, Wv, Wo = (np.asarray(w, np.float32) for w in (Wq, Wk, Wv, Wo))
    bo = np.asarray(bo, np.float32)
    Wg, Wf = np.asarray(Wg, np.float32), np.asarray(Wf, np.float32)
    bg, bf = np.asarray(bg, np.float32), np.asarray(bf, np.float32)

    use_gamma = not np.all(ln_gamma == 1.0)
    use_beta = not np.all(ln_beta == 0.0)
    use_bo = not np.all(bo == 0.0)
    use_bg = not np.all(bg == 0.0)
    use_bf = not np.all(bf == 0.0)
    flags = (use_gamma, use_beta, use_bo, use_bg, use_bf)

    if flags not in _CACHE:
        _CACHE[flags] = _build(flags)
    nc = _CACHE[flags]

    ws = WSCALE
    wq_h = np.stack([_swz(Wq[l] * ws, CCH) for l in range(L)]).astype(F8)
    wk_h = np.stack([_swz(Wk[l] * ws, CCH) for l in range(L)]).astype(F8)
    wv_h = np.stack([_swz(Wv[l] * ws, CCH) for l in range(L)]).astype(F8)
    wo_h = np.stack([_swz(Wo[l] * ws, CCH) for l in range(L)]).astype(F8)
    wg_h = _swz(Wg * ws, CCH).astype(F8)
    wf_h = _swz(Wf * ws, 2).astype(F8)

    xf = x.reshape(B * S, C)
    base = {
        "wq": wq_h, "wk": wk_h, "wv": wv_h, "wo": wo_h,
        "wg": wg_h, "wf": wf_h,
    }
    if use_gamma:
        base["gam"] = np.ascontiguousarray(
            np.broadcast_to(ln_gamma[:, None, :], (L + 1, P, C))).astype(BF16)
    if use_beta:
        base["bet"] = np.ascontiguousarray(
            np.broadcast_to(ln_beta[:, None, :], (L + 1, P, C))).astype(BF16)
    if use_bo:
        base["bob"] = np.ascontiguousarray(
            np.broadcast_to(bo[:, None, :], (L, P, C))).astype(np.float32)
    if use_bg:
        base["bgc"] = np.ascontiguousarray(bg.reshape(CCH, P).T).astype(np.float32)
    if use_bf:
        base["bfb"] = np.ascontiguousarray(
            np.broadcast_to(bf[None, :], (P, C))).astype(np.float32)

    in_maps = []
    for c in range(N_CORES):
        m = dict(base)
        m["x"] = np.ascontiguousarray(xf[c * ROWS:(c + 1) * ROWS])
        in_maps.append(m)

    from concourse.bass_utils import run_bass_kernel_spmd
    res = run_bass_kernel_spmd(nc, in_maps, core_ids=list(range(N_CORES)))
    out = np.concatenate([res.results[c]["y"] for c in range(N_CORES)], axis=0)
    return out.reshape(B, S, C).astype(np.float32)


# revision 33
# speedup vs baseline: 1.0499x; 1.0021x over previous
"""Trainium2 Bass kernel for a 4-layer pre-LN transformer + GEGLU FFN.

Sharding: rows (batch*seq) split across 8 cores; cores 0-3 own batch 0,
cores 4-7 own batch 1 (512 rows each).  Attention needs full-sequence K/V
per batch element, so each 4-core group AllGathers its K/V shards per layer.

Per core, per layer:
  LN (DVE stats + quake-rsqrt), h^T via DMA x-bar transpose, h^T also
  quantized to fp8 (HT8).
  All projections are fp8 DoubleRow matmuls (2 c-chunks contracted per
  instruction); weights are host-prescaled by 64 (sigma=0.02 would be
  denormal in e4m3) and the PSUM-evacuation op folds the 1/64 back in.
  K^T -> bounce -> AllGather (bf16).  V natural fp8 with a ones column per
  head -> AllGather (fp8 bytes in bf16-typed buffers).
  OWN-FIRST attention: scores/exp for the core's own 4 k-tiles (read from
  the pre-collective staging) run while the AllGathers are in flight; the
  3 remote blocks are unloaded with runtime-indexed DMAs (partition_id()
  picks the 3 non-own blocks) and processed after.
  Scores: head pairs in the two 64-row halves of the PE array, with the
  odd head STAGGERED one k-group behind the even head so both heads'
  matmuls are dependency-ready and actually overlap in the array.
  Exp on ACT in 3-PSUM-bank groups, writing fp8 exp-scores.
  AV: fp8 DoubleRow (2 k-tiles per matmul), ones column gives softmax
  sums for free; AV + normalization of the previous pair interleave into
  the score emission.  Normalization: ones-matmul broadcast of the sums
  then reciprocal_approx_fast at base partition 0 (the custom-DVE op
  mishandles nonzero base partitions).
Final LN + GEGLU FFN (fp8 DR matmuls, explicit tanh) + residual -> output.
"""

import numpy as np
import ml_dtypes

B, S, C = 2, 2048, 512
L, H, CH = 4, 8, 64
OD = 4 * CH  # 256
EPS = 1e-5

N_CORES = 8
GROUP = 4          # cores per batch element
ROWS = (B * S) // N_CORES  # 512 rows per core
P = 128
RT = ROWS // P     # 4 row tiles
CCH = C // P       # 4 chunks of the hidden/attention dim
KT = S // P        # 16 k tiles (full sequence)
KTO = ROWS // P    # 4 own k tiles
KTR = KT - KTO     # 12 remote k tiles
VW = H * 80        # fp8 V row layout: 64 data + 1 ones + 15 pad per head
SQRT_K = 0x5F3759DF
WSCALE = 64.0      # host-side fp8 weight prescale

BF16 = ml_dtypes.bfloat16
F8 = ml_dtypes.float8_e4m3fn

_CACHE = {}


def _swz(w, pt):
    # [pt*128, N] -> [128, pt*N] with chunk-major free dim
    n = w.shape[1]
    return np.ascontiguousarray(
        w.reshape(pt, P, n).transpose(1, 0, 2).reshape(P, pt * n)
    )


def _build(flags, n_layers=L, fake_ag=False):
    use_gamma, use_beta, use_bo, use_bg, use_bf = flags
    import concourse.bass as bass
    import concourse.bacc as bacc
    import concourse.mybir as mybir
    import concourse.tile as tile

    dt = mybir.dt
    OP = mybir.AluOpType
    AF = mybir.ActivationFunctionType
    DR = mybir.MatmulPerfMode.DoubleRow
    IWS = 1.0 / WSCALE

    nc = bacc.Bacc("TRN2", target_bir_lowering=False, debug=False,
                   num_devices=N_CORES)
    groups = [list(range(g * GROUP, (g + 1) * GROUP))
              for g in range(N_CORES // GROUP)]

    # ---- DRAM I/O (all weights fp8, prescaled by WSCALE on the host) ----
    x_d = nc.dram_tensor("x", [ROWS, C], dt.float32, kind="ExternalInput")
    wq_d = nc.dram_tensor("wq", [L, P, CCH * C], dt.float8e4, kind="ExternalInput")
    wk_d = nc.dram_tensor("wk", [L, P, CCH * C], dt.float8e4, kind="ExternalInput")
    wv_d = nc.dram_tensor("wv", [L, P, CCH * C], dt.float8e4, kind="ExternalInput")
    wo_d = nc.dram_tensor("wo", [L, P, CCH * C], dt.float8e4, kind="ExternalInput")
    wg_d = nc.dram_tensor("wg", [P, CCH * C], dt.float8e4, kind="ExternalInput")
    wf_d = nc.dram_tensor("wf", [P, 2 * C], dt.float8e4, kind="ExternalInput")
    y_d = nc.dram_tensor("y", [ROWS, C], dt.float32, kind="ExternalOutput")
    if use_gamma:
        gam_d = nc.dram_tensor("gam", [L + 1, P, C], dt.bfloat16, kind="ExternalInput")
    if use_beta:
        bet_d = nc.dram_tensor("bet", [L + 1, P, C], dt.bfloat16, kind="ExternalInput")
    if use_bo:
        bo_d = nc.dram_tensor("bob", [L, P, C], dt.float32, kind="ExternalInput")
    if use_bg:
        bg_d = nc.dram_tensor("bgc", [P, CCH], dt.float32, kind="ExternalInput")
    if use_bf:
        bf_d = nc.dram_tensor("bfb", [P, C], dt.float32, kind="ExternalInput")

    # separate K / V bounce buffers (fp8 bytes in bf16-typed buffers):
    # the small K AllGather launches right after the K projection so it
    # lands before the own-shard score phase runs out of work.
    KHALF = CCH * ROWS // 2
    VHALF = KTO * VW // 2
    kin_k = [nc.dram_tensor(f"kin_k{i}", [P, KHALF], dt.bfloat16,
                            kind="Internal") for i in range(2)]
    kout_k = [nc.dram_tensor(f"kout_k{i}", [GROUP, P, KHALF], dt.bfloat16,
                             kind="Internal") for i in range(2)]
    kin_v = [nc.dram_tensor(f"kin_v{i}", [P, VHALF], dt.bfloat16,
                            kind="Internal") for i in range(2)]
    kout_v = [nc.dram_tensor(f"kout_v{i}", [GROUP, P, VHALF], dt.bfloat16,
                             kind="Internal") for i in range(2)]

    # ---- persistent SBUF ----
    XSB = nc.alloc_sbuf_tensor("xsb", [P, RT * C], dt.float32).ap()
    HSB = nc.alloc_sbuf_tensor("hsb", [P, RT * C], dt.bfloat16).ap()
    HTSB = nc.alloc_sbuf_tensor("htsb", [P, CCH * ROWS], dt.bfloat16).ap()
    HT8 = nc.alloc_sbuf_tensor("ht8", [P, CCH, ROWS], dt.float8e4).ap()
    QTSB = nc.alloc_sbuf_tensor("qtsb", [P, CCH * ROWS], dt.bfloat16).ap()
    KSTG = nc.alloc_sbuf_tensor("kstg", [P, CCH * ROWS], dt.bfloat16).ap()
    K8STG = nc.alloc_sbuf_tensor("k8stg", [P, CCH * ROWS], dt.float8e4).ap()
    KT8 = nc.alloc_sbuf_tensor("kt8", [P, CCH * KTR * P], dt.float8e4).ap()
    KTSB = nc.alloc_sbuf_tensor("ktsb", [P, CCH * KTR * P], dt.bfloat16).ap()
    VSTG = nc.alloc_sbuf_tensor("vstg", [P, KTO, H, 80], dt.float8e4).ap()
    VSB = nc.alloc_sbuf_tensor("vsb", [P, KTR, H, 80], dt.float8e4).ap()
    OSC = nc.alloc_sbuf_tensor("osc", [P, CCH, ROWS], dt.float8e4).ap()
    FFSB = nc.alloc_sbuf_tensor("ffsb", [P, 2, ROWS], dt.float8e4).ap()
    ONES = nc.alloc_sbuf_tensor("ones", [P, P], dt.bfloat16).ap()
    RECB = nc.alloc_sbuf_tensor("recb", [P, 2 * ROWS], dt.bfloat16).ap()

    with tile.TileContext(nc) as tc:
        with (
            tc.tile_pool(name="wpool", bufs=2) as wpool,
            tc.tile_pool(name="epool", bufs=4) as epool,
            tc.tile_pool(name="small", bufs=2) as small,
            tc.tile_pool(name="gpool", bufs=2) as gpool,
            tc.tile_pool(name="mmps", bufs=2, space="PSUM") as mmps,
            tc.tile_pool(name="scps", bufs=2, space="PSUM") as scps,
        ):
            # one-time init
            nc.vector.memset(ONES, 1.0)
            nc.vector.memset(VSTG.rearrange("p a b c -> p (a b c)"), 0.0)
            nc.vector.memset(VSTG[:, :, :, 64:65], 1.0)
            nc.sync.dma_start(XSB.rearrange("p (rt c) -> p rt c", rt=RT),
                              x_d.ap().rearrange("(rt p) c -> p rt c", p=P))

            # runtime rank within the replica group (for remote-block DMAs)
            me = nc.sync.partition_id() % GROUP

            def layer_norm(li):
                """x (XSB) -> h bf16 (HSB), h^T (HTSB) + fp8 h^T (HT8)."""
                MV = small.tile([P, 2 * RT], dt.float32, tag="mv")
                for rt in range(RT):
                    st6 = small.tile([P, 6], dt.float32, tag="st6")
                    nc.vector.bn_stats(st6[:], XSB[:, rt * C:(rt + 1) * C])
                    nc.vector.bn_aggr(MV[:, 2 * rt:2 * rt + 2], st6[:])
                var = MV[:].rearrange("p (rt two) -> p two rt", two=2)[:, 1, :]
                VT = small.tile([P, RT], dt.float32, tag="vt")
                VH = small.tile([P, RT], dt.float32, tag="vh")
                KC = small.tile([P, RT], dt.int32, tag="kc")
                R0 = small.tile([P, RT], dt.int32, tag="r0")
                nc.vector.tensor_scalar(VT[:], var, EPS, None, OP.add)
                nc.vector.tensor_scalar(VH[:], VT[:], 0.5, None, OP.mult)
                nc.vector.memset(KC[:], SQRT_K)
                nc.vector.tensor_scalar(R0[:], VT[:].bitcast(dt.int32), 1, None,
                                        OP.logical_shift_right)
                nc.vector.scalar_tensor_tensor(R0[:], KC[:], 0, R0[:],
                                               OP.bypass, OP.subtract)
                r = R0[:].bitcast(dt.float32)
                for _ in range(2):
                    A = small.tile([P, RT], dt.float32, tag="nra")
                    Cc = small.tile([P, RT], dt.float32, tag="nrc")
                    Rn = small.tile([P, RT], dt.float32, tag="nrr")
                    nc.vector.tensor_mul(A[:], r, r)
                    nc.vector.tensor_mul(A[:], A[:], VH[:])
                    nc.vector.tensor_scalar(Cc[:], A[:], -1.0, 1.5, OP.mult, OP.add)
                    nc.vector.tensor_mul(Rn[:], r, Cc[:])
                    r = Rn[:]
                if use_gamma:
                    GT = gpool.tile([P, C], dt.bfloat16, tag="gam")
                    nc.sync.dma_start(GT[:], gam_d.ap()[li])
                if use_beta:
                    BT = gpool.tile([P, C], dt.bfloat16, tag="bet")
                    nc.sync.dma_start(BT[:], bet_d.ap()[li])
                htv = HTSB.rearrange("p (cc r) -> p cc r", cc=CCH)
                for rt in range(RT):
                    dst = HSB[:, rt * C:(rt + 1) * C]
                    nc.vector.tensor_scalar(dst, XSB[:, rt * C:(rt + 1) * C],
                                            MV[:, 2 * rt:2 * rt + 1],
                                            r[:, rt:rt + 1],
                                            OP.subtract, OP.mult)
                    if use_gamma:
                        nc.vector.tensor_mul(dst, dst, GT[:])
                    if use_beta:
                        nc.vector.tensor_add(dst, dst, BT[:])
                    nc.sync.dma_start_transpose(
                        htv[:, :, rt * P:(rt + 1) * P],
                        HSB[:, rt * C:(rt + 1) * C])
                nc.vector.tensor_copy(HT8.rearrange("p a b -> p (a b)"), HTSB)

            def proj_dr(w8, dst_col):
                """fp8 DoubleRow transposed projection -> [c_out, rows]."""
                wv8 = w8.rearrange("p (cc c) -> p cc c", cc=CCH)
                for mc in range(CCH):
                    ps = mmps.tile([P, ROWS], dt.float32, tag="mm")
                    for kp in range(CCH // 2):
                        nc.tensor.matmul(
                            ps[:],
                            lhsT=wv8[:, 2 * kp:2 * kp + 2, mc * P:(mc + 1) * P],
                            rhs=HT8[:, 2 * kp:2 * kp + 2, :],
                            start=(kp == 0), stop=(kp == CCH // 2 - 1),
                            perf_mode=DR)
                    dst_col(mc, ps)

            def attn_layer(li):
                WQ = wpool.tile([P, CCH * C], dt.float8e4, tag="wq")
                WK = wpool.tile([P, CCH * C], dt.float8e4, tag="wk")
                WV = wpool.tile([P, CCH * C], dt.float8e4, tag="wv")
                WO = wpool.tile([P, CCH * C], dt.float8e4, tag="wo")
                nc.sync.dma_start(WK[:], wk_d.ap()[li])
                nc.sync.dma_start(WV[:], wv_d.ap()[li])
                nc.sync.dma_start(WQ[:], wq_d.ap()[li])
                nc.sync.dma_start(WO[:], wo_d.ap()[li])

                layer_norm(li)

                kin_ki, kout_ki = kin_k[li % 2], kout_k[li % 2]
                kin_vi, kout_vi = kin_v[li % 2], kout_v[li % 2]

                # K^T (own shard): bf16 for own scores + fp8 for transport
                def k_out(mc, ps):
                    nc.vector.tensor_scalar(
                        KSTG[:, mc * ROWS:(mc + 1) * ROWS], ps[:], IWS, None,
                        OP.mult)
                    nc.vector.tensor_scalar(
                        K8STG[:, mc * ROWS:(mc + 1) * ROWS], ps[:], IWS, None,
                        OP.mult)
                proj_dr(WK[:], k_out)
                nc.sync.dma_start(kin_ki.ap(), K8STG.bitcast(dt.bfloat16))
                if fake_ag:
                    for r in range(GROUP):
                        nc.sync.dma_start(kout_ki.ap()[r], kin_ki.ap())
                else:
                    nc.gpsimd.collective_compute(
                        "AllGather", mybir.AluOpType.bypass, replica_groups=groups,
                        ins=[kin_ki.ap().opt()], outs=[kout_ki.ap().opt()])

                # Q^T first: the own-shard score phase needs it, so it
                # starts earlier and covers more of the K AllGather flight
                proj_dr(WQ[:], lambda mc, ps: nc.vector.tensor_scalar(
                    QTSB[:, mc * ROWS:(mc + 1) * ROWS], ps[:], IWS, None, OP.mult))

                # V (own shard, natural, fp8, ones col)
                wv8 = WV[:].rearrange("p (cc c) -> p cc c", cc=CCH)
                for kt in range(KTO):
                    ps = mmps.tile([P, C], dt.float32, tag="mm")
                    for kp in range(CCH // 2):
                        nc.tensor.matmul(
                            ps[:],
                            lhsT=HT8[:, 2 * kp:2 * kp + 2, kt * P:(kt + 1) * P],
                            rhs=wv8[:, 2 * kp:2 * kp + 2, :],
                            start=(kp == 0), stop=(kp == CCH // 2 - 1),
                            perf_mode=DR)
                    nc.vector.tensor_scalar(
                        VSTG[:, kt, :, 0:CH],
                        ps[:].rearrange("p (h c) -> p h c", h=H), IWS, None, OP.mult)
                nc.sync.dma_start(
                    kin_vi.ap(),
                    VSTG.rearrange("p a b c -> p (a b c)").bitcast(dt.bfloat16))
                if fake_ag:
                    for r in range(GROUP):
                        nc.sync.dma_start(kout_vi.ap()[r], kin_vi.ap())
                else:
                    nc.gpsimd.collective_compute(
                        "AllGather", mybir.AluOpType.bypass, replica_groups=groups,
                        ins=[kin_vi.ap().opt()], outs=[kout_vi.ap().opt()])

                # unload the 3 REMOTE blocks (runtime-indexed by rank)
                if fake_ag:
                    srcs_k = [kout_ki.ap()[j] for j in range(1, GROUP)]
                    srcs_v = [kout_vi.ap()[j] for j in range(1, GROUP)]
                else:
                    srcs_k = [kout_ki.ap()[(me + j) % GROUP] for j in range(1, GROUP)]
                    srcs_v = [kout_vi.ap()[(me + j) % GROUP] for j in range(1, GROUP)]
                kt8v = KT8.rearrange("p (cc k) -> p cc k", cc=CCH)
                ktbv = KTSB.rearrange("p (cc k) -> p cc k", cc=CCH)
                for j in range(1, GROUP):
                    nc.sync.dma_start(
                        kt8v[:, :, (j - 1) * ROWS:j * ROWS].bitcast(dt.bfloat16),
                        srcs_k[j - 1].rearrange("p (cc r) -> p cc r", cc=CCH))
                    # upcast this block to bf16 for the score matmuls
                    nc.vector.tensor_copy(
                        ktbv[:, :, (j - 1) * ROWS:j * ROWS],
                        kt8v[:, :, (j - 1) * ROWS:j * ROWS])
                vsbf = VSB.rearrange("p a b c -> p (a b c)").bitcast(dt.bfloat16)
                for j in range(1, GROUP):
                    nc.sync.dma_start(vsbf[:, (j - 1) * VHALF:j * VHALF],
                                      srcs_v[j - 1])

                # ---- attention ----
                # esb slot s holds k-tile s: slots 0-3 own (KSTG/VSTG),
                # slots 4-15 remote (KTSB/VSB in arrival order).
                OG = [[0, 1, 2], [3]]                 # own k-slot groups
                RG = [[4, 5, 6], [7, 8, 9], [10, 11, 12], [13, 14, 15]]

                def score_mm(hp, sub, ps, j, slot):
                    if slot < KTO:
                        lt = KSTG[sub:sub + CH,
                                  hp * ROWS + slot * P: hp * ROWS + (slot + 1) * P]
                    else:
                        rk = slot - KTO
                        lt = KTSB[sub:sub + CH,
                                  hp * KTR * P + rk * P: hp * KTR * P + (rk + 1) * P]
                    nc.tensor.matmul(
                        ps[:, j * ROWS:(j + 1) * ROWS], lhsT=lt,
                        rhs=QTSB[sub:sub + CH, hp * ROWS:(hp + 1) * ROWS],
                        start=True, stop=True)

                def emit_groups(hp, esA, esB, glist, drain):
                    """staggered A/B score+exp emission over glist."""
                    for gi in range(len(glist) + 1):
                        todo = []
                        if gi >= 1:
                            psB = scps.tile([P, 3 * ROWS], dt.float32, tag="sc")
                            todo.append((CH, psB, esB, glist[gi - 1]))
                        if gi < len(glist):
                            psA = scps.tile([P, 3 * ROWS], dt.float32, tag="sc")
                            todo.append((0, psA, esA, glist[gi]))
                        for j in range(max(len(t[3]) for t in todo)):
                            for sub, ps, es, kts in todo:
                                if j < len(kts):
                                    score_mm(hp, sub, ps, j, kts[j])
                        for sub, ps, es, kts in todo:
                            nc.scalar.activation(
                                es[:, kts[0]:kts[0] + len(kts), :],
                                ps[:, 0:len(kts) * ROWS],
                                AF.Exp, scale=1.0 / np.sqrt(CH))
                        if drain:
                            for _ in range(4):
                                if drain:
                                    drain.pop(0)()

                def make_pair_work(hp, esA, esB):
                    work = []
                    avA = mmps.tile([P, ROWS], dt.float32, tag="mm")
                    avB = mmps.tile([P, ROWS], dt.float32, tag="mm")
                    for t in range(KT // 2):
                        for av, es in ((avA, esA), (avB, esB)):
                            h = 2 * hp + (0 if av is avA else 1)
                            if t < KTO // 2:
                                lt = VSTG[:, 2 * t:2 * t + 2, h, 0:80]
                            else:
                                rt_ = t - KTO // 2
                                lt = VSB[:, 2 * rt_:2 * rt_ + 2, h, 0:80]
                            work.append((lambda av=av, es=es, t=t, lt=lt: nc.tensor.matmul(
                                av[0:80, :], lhsT=lt,
                                rhs=es[:, 2 * t:2 * t + 2, :],
                                start=(t == 0), stop=(t == KT // 2 - 1),
                                perf_mode=DR)))

                    def post_head(j, av):
                        h = 2 * hp + j
                        sub = (h % 2) * CH
                        nc.vector.tensor_copy(
                            OSC[sub:sub + CH, hp, :], av[0:CH, :])
                        nc.vector.tensor_copy(
                            RECB[CH:CH + 1, j * ROWS:(j + 1) * ROWS],
                            av[CH:CH + 1, :])
                        bc = mmps.tile([P, ROWS], dt.float32, tag="mm")
                        nc.tensor.matmul(
                            bc[:, :], lhsT=ONES[CH:CH + 1, :],
                            rhs=RECB[CH:CH + 1, j * ROWS:(j + 1) * ROWS],
                            start=True, stop=True)
                        rb = small.tile([P, ROWS], dt.float32, tag="rb")
                        nc.vector.reciprocal_approx_fast(rb[:], bc[:, :])
                        dst = OSC[sub:sub + CH, hp, :]
                        nc.vector.tensor_mul(dst, dst, rb[sub:sub + CH, :])
                    work.append(lambda: post_head(0, avA))
                    work.append(lambda: post_head(1, avB))
                    return work

                # phase A: own k-tiles for all pairs (overlaps the AllGathers)
                esbs = []
                for hp in range(H // 2):
                    esA = epool.tile([P, KT, ROWS], dt.float8e4, tag="esA")
                    esB = epool.tile([P, KT, ROWS], dt.float8e4, tag="esB")
                    esbs.append((esA, esB))
                    emit_groups(hp, esA, esB, OG, None)
                # phase B: remote k-tiles; AV of the previous pair interleaves
                pending = []
                for hp in range(H // 2):
                    esA, esB = esbs[hp]
                    emit_groups(hp, esA, esB, RG, pending)
                    while pending:
                        pending.pop(0)()
                    pending = make_pair_work(hp, esA, esB)
                while pending:
                    pending.pop(0)()

                # out-projection (fp8 DR) + residual
                if use_bo:
                    BO = gpool.tile([P, C], dt.float32, tag="bo")
                    nc.sync.dma_start(BO[:], bo_d.ap()[li])
                wo8 = WO[:].rearrange("p (cc c) -> p cc c", cc=CCH)
                for rt in range(RT):
                    ps = mmps.tile([P, C], dt.float32, tag="mm")
                    for kp in range(CCH // 2):
                        nc.tensor.matmul(
                            ps[:],
                            lhsT=OSC[:, 2 * kp:2 * kp + 2, rt * P:(rt + 1) * P],
                            rhs=wo8[:, 2 * kp:2 * kp + 2, :],
                            start=(kp == 0), stop=(kp == CCH // 2 - 1),
                            perf_mode=DR)
                    dst = XSB[:, rt * C:(rt + 1) * C]
                    nc.vector.scalar_tensor_tensor(dst, ps[:], IWS, dst,
                                                   OP.mult, OP.add)
                    if use_bo:
                        nc.vector.tensor_add(dst, dst, BO[:])

            for li in range(n_layers):
                attn_layer(li)

            # ---- FFN (fp8 DR) ----
            layer_norm(L)
            WG = wpool.tile([P, CCH * C], dt.float8e4, tag="wq")
            WF = wpool.tile([P, 2 * C], dt.float8e4, tag="wf")
            nc.sync.dma_start(WG[:], wg_d.ap())
            nc.sync.dma_start(WF[:], wf_d.ap())
            if use_bg:
                BG = gpool.tile([P, CCH], dt.float32, tag="bg")
                nc.sync.dma_start(BG[:], bg_d.ap())
            AGT = small.tile([P, 2 * ROWS], dt.bfloat16, tag="ffa")
            GGT = small.tile([P, 2 * ROWS], dt.bfloat16, tag="ffg")
            wg8 = WG[:].rearrange("p (cc c) -> p cc c", cc=CCH)
            for mg in range(CCH):
                ps = mmps.tile([P, ROWS], dt.float32, tag="mm")
                for kp in range(CCH // 2):
                    nc.tensor.matmul(
                        ps[:],
                        lhsT=wg8[:, 2 * kp:2 * kp + 2, mg * P:(mg + 1) * P],
                        rhs=HT8[:, 2 * kp:2 * kp + 2, :],
                        start=(kp == 0), stop=(kp == CCH // 2 - 1),
                        perf_mode=DR)
                dst = (AGT if mg < 2 else GGT)[:, (mg % 2) * ROWS:(mg % 2 + 1) * ROWS]
                if use_bg:
                    nc.vector.tensor_scalar(dst, ps[:], IWS, BG[:, mg:mg + 1],
                                            OP.mult, OP.add)
                else:
                    nc.vector.tensor_scalar(dst, ps[:], IWS, None, OP.mult)
            K1 = 0.7978845608
            for j in range(2):
                ga = GGT[:, j * ROWS:(j + 1) * ROWS]
                aa = AGT[:, j * ROWS:(j + 1) * ROWS]
                SQ = small.tile([P, ROWS], dt.float32, tag="sq")
                WT = small.tile([P, ROWS], dt.float32, tag="wt")
                VV = small.tile([P, ROWS], dt.float32, tag="vv")
                TT = small.tile([P, ROWS], dt.float32, tag="tt")
                HT2 = small.tile([P, ROWS], dt.bfloat16, tag="ht2")
                PP = small.tile([P, ROWS], dt.bfloat16, tag="pp")
                nc.scalar.activation(SQ[:], ga, AF.Square)
                nc.vector.tensor_scalar(WT[:], SQ[:], K1 * 0.044715, K1,
                                        OP.mult, OP.add)
                nc.vector.tensor_mul(VV[:], ga, WT[:])
                nc.scalar.activation(TT[:], VV[:], AF.Tanh)
                nc.vector.tensor_scalar(HT2[:], TT[:], 0.5, 0.5,
                                        OP.mult, OP.add)
                nc.vector.tensor_mul(PP[:], aa, ga)
                nc.vector.tensor_mul(FFSB[:, j, :], PP[:], HT2[:])
            if use_bf:
                BF = gpool.tile([P, C], dt.float32, tag="bf")
                nc.sync.dma_start(BF[:], bf_d.ap())
            wf8 = WF[:].rearrange("p (two c) -> p two c", two=2)
            for rt in range(RT):
                ps = mmps.tile([P, C], dt.float32, tag="mm")
                nc.tensor.matmul(
                    ps[:],
                    lhsT=FFSB[:, 0:2, rt * P:(rt + 1) * P],
                    rhs=wf8[:, 0:2, :],
                    start=True, stop=True, perf_mode=DR)
                OUT = small.tile([P, C], dt.float32, tag="out")
                nc.vector.scalar_tensor_tensor(OUT[:], ps[:], IWS,
                                               XSB[:, rt * C:(rt + 1) * C],
                                               OP.mult, OP.add)
                if use_bf:
                    nc.vector.tensor_add(OUT[:], OUT[:], BF[:])
                nc.sync.dma_start(y_d.ap()[rt * P:(rt + 1) * P, :], OUT[:])

    nc.compile()
    return nc


def kernel(x, ln_gamma, ln_beta, Wq, Wk, Wv, Wo, bo, Wg, bg, Wf, bf):
    x = np.asarray(x, np.float32)
    ln_gamma = np.asarray(ln_gamma, np.float32)
    ln_beta = np.asarray(ln_beta, np.float32)
    Wq, Wk, Wv, Wo = (np.asarray(w, np.float32) for w in (Wq, Wk, Wv, Wo))
    bo = np.asarray(bo, np.float32)
    Wg, Wf = np.asarray(Wg, np.float32), np.asarray(Wf, np.float32)
    bg, bf = np.asarray(bg, np.float32), np.asarray(bf, np.float32)

    use_gamma = not np.all(ln_gamma == 1.0)
    use_beta = not np.all(ln_beta == 0.0)
    use_bo = not np.all(bo == 0.0)
    use_bg = not np.all(bg == 0.0)
    use_bf = not np.all(bf == 0.0)
    flags = (use_gamma, use_beta, use_bo, use_bg, use_bf)

    if flags not in _CACHE:
        _CACHE[flags] = _build(flags)
    nc = _CACHE[flags]

    ws = WSCALE
    wq_h = np.stack([_swz(Wq[l] * ws, CCH) for l in range(L)]).astype(F8)
    wk_h = np.stack([_swz(Wk[l] * ws, CCH) for l in range(L)]).astype(F8)
    wv_h = np.stack([_swz(Wv[l] * ws, CCH) for l in range(L)]).astype(F8)
    wo_h = np.stack([_swz(Wo[l] * ws, CCH) for l in range(L)]).astype(F8)
    wg_h = _swz(Wg * ws, CCH).astype(F8)
    wf_h = _swz(Wf * ws, 2).astype(F8)

    xf = x.reshape(B * S, C)
    base = {
        "wq": wq_h, "wk": wk_h, "wv": wv_h, "wo": wo_h,
        "wg": wg_h, "wf": wf_h,
    }
    if use_gamma:
        base["gam"] = np.ascontiguousarray(
            np.broadcast_to(ln_gamma[:, None, :], (L + 1, P, C))).astype(BF16)
    if use_beta:
        base["bet"] = np.ascontiguousarray(
            np.broadcast_to(ln_beta[:, None, :], (L + 1, P, C))).astype(BF16)
    if use_bo:
        base["bob"] = np.ascontiguousarray(
            np.broadcast_to(bo[:, None, :], (L, P, C))).astype(np.float32)
    if use_bg:
        base["bgc"] = np.ascontiguousarray(bg.reshape(CCH, P).T).astype(np.float32)
    if use_bf:
        base["bfb"] = np.ascontiguousarray(
            np.broadcast_to(bf[None, :], (P, C))).astype(np.float32)

    in_maps = []
    for c in range(N_CORES):
        m = dict(base)
        m["x"] = np.ascontiguousarray(xf[c * ROWS:(c + 1) * ROWS])
        in_maps.append(m)

    from concourse.bass_utils import run_bass_kernel_spmd
    res = run_bass_kernel_spmd(nc, in_maps, core_ids=list(range(N_CORES)))
    out = np.concatenate([res.results[c]["y"] for c in range(N_CORES)], axis=0)
    return out.reshape(B, S, C).astype(np.float32)


# revision 34
# speedup vs baseline: 1.0719x; 1.0209x over previous
"""Trainium2 Bass kernel for a 4-layer pre-LN transformer + GEGLU FFN.

Sharding: rows (batch*seq) split across 8 cores; cores 0-3 own batch 0,
cores 4-7 own batch 1 (512 rows each).  Attention needs full-sequence K/V
per batch element, so each 4-core group AllGathers its K/V shards per layer.

Per core, per layer:
  LN (DVE stats + quake-rsqrt), h^T via DMA x-bar transpose, h^T also
  quantized to fp8 (HT8).
  All projections are fp8 DoubleRow matmuls (2 c-chunks contracted per
  instruction); weights are host-prescaled by 64 (sigma=0.02 would be
  denormal in e4m3) and the PSUM-evacuation op folds the 1/64 back in.
  K^T -> bounce -> AllGather (bf16).  V natural fp8 with a ones column per
  head -> AllGather (fp8 bytes in bf16-typed buffers).
  OWN-FIRST attention: scores/exp for the core's own 4 k-tiles (read from
  the pre-collective staging) run while the AllGathers are in flight; the
  3 remote blocks are unloaded with runtime-indexed DMAs (partition_id()
  picks the 3 non-own blocks) and processed after.
  Scores: head pairs in the two 64-row halves of the PE array, with the
  odd head STAGGERED one k-group behind the even head so both heads'
  matmuls are dependency-ready and actually overlap in the array.
  Exp on ACT in 3-PSUM-bank groups, writing fp8 exp-scores.
  AV: fp8 DoubleRow (2 k-tiles per matmul), ones column gives softmax
  sums for free; AV + normalization of the previous pair interleave into
  the score emission.  Normalization: ones-matmul broadcast of the sums
  then reciprocal_approx_fast at base partition 0 (the custom-DVE op
  mishandles nonzero base partitions).
Final LN + GEGLU FFN (fp8 DR matmuls, explicit tanh) + residual -> output.
"""

import numpy as np
import ml_dtypes

B, S, C = 2, 2048, 512
L, H, CH = 4, 8, 64
OD = 4 * CH  # 256
EPS = 1e-5

N_CORES = 8
GROUP = 4          # cores per batch element
ROWS = (B * S) // N_CORES  # 512 rows per core
P = 128
RT = ROWS // P     # 4 row tiles
CCH = C // P       # 4 chunks of the hidden/attention dim
KT = S // P        # 16 k tiles (full sequence)
KTO = ROWS // P    # 4 own k tiles
KTR = KT - KTO     # 12 remote k tiles
VW = H * 80        # fp8 V row layout: 64 data + 1 ones + 15 pad per head
SQRT_K = 0x5F3759DF
WSCALE = 64.0      # host-side fp8 weight prescale

BF16 = ml_dtypes.bfloat16
F8 = ml_dtypes.float8_e4m3fn

_CACHE = {}


def _swz(w, pt):
    # [pt*128, N] -> [128, pt*N] with chunk-major free dim
    n = w.shape[1]
    return np.ascontiguousarray(
        w.reshape(pt, P, n).transpose(1, 0, 2).reshape(P, pt * n)
    )


def _build(flags, n_layers=L, fake_ag=False):
    use_gamma, use_beta, use_bo, use_bg, use_bf = flags
    import concourse.bass as bass
    import concourse.bacc as bacc
    import concourse.mybir as mybir
    import concourse.tile as tile

    dt = mybir.dt
    OP = mybir.AluOpType
    AF = mybir.ActivationFunctionType
    DR = mybir.MatmulPerfMode.DoubleRow
    IWS = 1.0 / WSCALE

    nc = bacc.Bacc("TRN2", target_bir_lowering=False, debug=False,
                   num_devices=N_CORES)
    groups = [list(range(g * GROUP, (g + 1) * GROUP))
              for g in range(N_CORES // GROUP)]

    # ---- DRAM I/O (all weights fp8, prescaled by WSCALE on the host) ----
    x_d = nc.dram_tensor("x", [ROWS, C], dt.float32, kind="ExternalInput")
    wq_d = nc.dram_tensor("wq", [L, P, CCH * C], dt.float8e4, kind="ExternalInput")
    wk_d = nc.dram_tensor("wk", [L, P, CCH * C], dt.float8e4, kind="ExternalInput")
    wv_d = nc.dram_tensor("wv", [L, P, CCH * C], dt.float8e4, kind="ExternalInput")
    wo_d = nc.dram_tensor("wo", [L, P, CCH * C], dt.float8e4, kind="ExternalInput")
    wg_d = nc.dram_tensor("wg", [P, CCH * C], dt.float8e4, kind="ExternalInput")
    wf_d = nc.dram_tensor("wf", [P, 2 * C], dt.float8e4, kind="ExternalInput")
    y_d = nc.dram_tensor("y", [ROWS, C], dt.float32, kind="ExternalOutput")
    if use_gamma:
        gam_d = nc.dram_tensor("gam", [L + 1, P, C], dt.bfloat16, kind="ExternalInput")
    if use_beta:
        bet_d = nc.dram_tensor("bet", [L + 1, P, C], dt.bfloat16, kind="ExternalInput")
    if use_bo:
        bo_d = nc.dram_tensor("bob", [L, P, C], dt.float32, kind="ExternalInput")
    if use_bg:
        bg_d = nc.dram_tensor("bgc", [P, CCH], dt.float32, kind="ExternalInput")
    if use_bf:
        bf_d = nc.dram_tensor("bfb", [P, C], dt.float32, kind="ExternalInput")

    # separate K / V bounce buffers (fp8 bytes in bf16-typed buffers):
    # the small K AllGather launches right after the K projection so it
    # lands before the own-shard score phase runs out of work.
    KHALF = CCH * ROWS // 2
    VHALF = KTO * VW // 2
    kin_k = [nc.dram_tensor(f"kin_k{i}", [P, KHALF], dt.bfloat16,
                            kind="Internal") for i in range(2)]
    kout_k = [nc.dram_tensor(f"kout_k{i}", [GROUP, P, KHALF], dt.bfloat16,
                             kind="Internal") for i in range(2)]
    kin_v = [nc.dram_tensor(f"kin_v{i}", [P, VHALF], dt.bfloat16,
                            kind="Internal") for i in range(2)]
    kout_v = [nc.dram_tensor(f"kout_v{i}", [GROUP, P, VHALF], dt.bfloat16,
                             kind="Internal") for i in range(2)]

    # ---- persistent SBUF ----
    XSB = nc.alloc_sbuf_tensor("xsb", [P, RT * C], dt.float32).ap()
    HSB = nc.alloc_sbuf_tensor("hsb", [P, RT * C], dt.bfloat16).ap()
    HTSB = nc.alloc_sbuf_tensor("htsb", [P, CCH * ROWS], dt.bfloat16).ap()
    HT8 = nc.alloc_sbuf_tensor("ht8", [P, CCH, ROWS], dt.float8e4).ap()
    QTSB = nc.alloc_sbuf_tensor("qtsb", [P, CCH * ROWS], dt.bfloat16).ap()
    KSTG = nc.alloc_sbuf_tensor("kstg", [P, CCH * ROWS], dt.bfloat16).ap()
    K8STG = nc.alloc_sbuf_tensor("k8stg", [P, CCH * ROWS], dt.float8e4).ap()
    KT8 = nc.alloc_sbuf_tensor("kt8", [P, CCH * KTR * P], dt.float8e4).ap()
    KTSB = nc.alloc_sbuf_tensor("ktsb", [P, CCH * KTR * P], dt.bfloat16).ap()
    VSTG = nc.alloc_sbuf_tensor("vstg", [P, KTO, H, 80], dt.float8e4).ap()
    VSB = nc.alloc_sbuf_tensor("vsb", [P, KTR, H, 80], dt.float8e4).ap()
    OSC = nc.alloc_sbuf_tensor("osc", [P, CCH, ROWS], dt.float8e4).ap()
    FFSB = nc.alloc_sbuf_tensor("ffsb", [P, 2, ROWS], dt.float8e4).ap()
    ONES = nc.alloc_sbuf_tensor("ones", [P, P], dt.bfloat16).ap()
    RECB = nc.alloc_sbuf_tensor("recb", [P, 2 * ROWS], dt.bfloat16).ap()

    with tile.TileContext(nc) as tc:
        with (
            tc.tile_pool(name="wpool", bufs=2) as wpool,
            tc.tile_pool(name="epool", bufs=4) as epool,
            tc.tile_pool(name="small", bufs=2) as small,
            tc.tile_pool(name="gpool", bufs=2) as gpool,
            tc.tile_pool(name="mmps", bufs=2, space="PSUM") as mmps,
            tc.tile_pool(name="scps", bufs=2, space="PSUM") as scps,
        ):
            # one-time init
            nc.vector.memset(ONES, 1.0)
            nc.vector.memset(VSTG.rearrange("p a b c -> p (a b c)"), 0.0)
            nc.vector.memset(VSTG[:, :, :, 64:65], 1.0)
            nc.sync.dma_start(XSB.rearrange("p (rt c) -> p rt c", rt=RT),
                              x_d.ap().rearrange("(rt p) c -> p rt c", p=P))

            # runtime rank within the replica group (for remote-block DMAs)
            me = nc.sync.partition_id() % GROUP

            def layer_norm(li):
                """x (XSB) -> h bf16 (HSB), h^T (HTSB) + fp8 h^T (HT8)."""
                MV = small.tile([P, 2 * RT], dt.float32, tag="mv")
                for rt in range(RT):
                    st6 = small.tile([P, 6], dt.float32, tag="st6")
                    nc.vector.bn_stats(st6[:], XSB[:, rt * C:(rt + 1) * C])
                    nc.vector.bn_aggr(MV[:, 2 * rt:2 * rt + 2], st6[:])
                var = MV[:].rearrange("p (rt two) -> p two rt", two=2)[:, 1, :]
                VT = small.tile([P, RT], dt.float32, tag="vt")
                VH = small.tile([P, RT], dt.float32, tag="vh")
                KC = small.tile([P, RT], dt.int32, tag="kc")
                R0 = small.tile([P, RT], dt.int32, tag="r0")
                nc.vector.tensor_scalar(VT[:], var, EPS, None, OP.add)
                nc.vector.tensor_scalar(VH[:], VT[:], 0.5, None, OP.mult)
                nc.vector.memset(KC[:], SQRT_K)
                nc.vector.tensor_scalar(R0[:], VT[:].bitcast(dt.int32), 1, None,
                                        OP.logical_shift_right)
                nc.vector.scalar_tensor_tensor(R0[:], KC[:], 0, R0[:],
                                               OP.bypass, OP.subtract)
                r = R0[:].bitcast(dt.float32)
                for _ in range(2):
                    A = small.tile([P, RT], dt.float32, tag="nra")
                    Cc = small.tile([P, RT], dt.float32, tag="nrc")
                    Rn = small.tile([P, RT], dt.float32, tag="nrr")
                    nc.vector.tensor_mul(A[:], r, r)
                    nc.vector.tensor_mul(A[:], A[:], VH[:])
                    nc.vector.tensor_scalar(Cc[:], A[:], -1.0, 1.5, OP.mult, OP.add)
                    nc.vector.tensor_mul(Rn[:], r, Cc[:])
                    r = Rn[:]
                if use_gamma:
                    GT = gpool.tile([P, C], dt.bfloat16, tag="gam")
                    nc.sync.dma_start(GT[:], gam_d.ap()[li])
                if use_beta:
                    BT = gpool.tile([P, C], dt.bfloat16, tag="bet")
                    nc.sync.dma_start(BT[:], bet_d.ap()[li])
                # split the normalize across DVE and the (idle) ACT
                # engine: Identity(x*rstd + (-mu*rstd)) == (x - mu)*rstd
                mu_v = MV[:].rearrange("p (rt two) -> p two rt", two=2)[:, 0, :]
                NB = small.tile([P, RT], dt.float32, tag="nb")
                nc.vector.scalar_tensor_tensor(NB[:], mu_v, -1.0, r,
                                               OP.mult, OP.mult)
                htv = HTSB.rearrange("p (cc r) -> p cc r", cc=CCH)
                for rt in range(RT):
                    dst = HSB[:, rt * C:(rt + 1) * C]
                    if rt % 2 == 0:
                        nc.vector.tensor_scalar(dst,
                                                XSB[:, rt * C:(rt + 1) * C],
                                                MV[:, 2 * rt:2 * rt + 1],
                                                r[:, rt:rt + 1],
                                                OP.subtract, OP.mult)
                    else:
                        nc.scalar.activation(dst, XSB[:, rt * C:(rt + 1) * C],
                                             AF.Identity,
                                             bias=NB[:, rt:rt + 1],
                                             scale=r[:, rt:rt + 1])
                    if use_gamma:
                        nc.vector.tensor_mul(dst, dst, GT[:])
                    if use_beta:
                        nc.vector.tensor_add(dst, dst, BT[:])
                    nc.sync.dma_start_transpose(
                        htv[:, :, rt * P:(rt + 1) * P],
                        HSB[:, rt * C:(rt + 1) * C])
                nc.vector.tensor_copy(HT8.rearrange("p a b -> p (a b)"), HTSB)

            def proj_dr(w8, dst_col):
                """fp8 DoubleRow transposed projection -> [c_out, rows]."""
                wv8 = w8.rearrange("p (cc c) -> p cc c", cc=CCH)
                for mc in range(CCH):
                    ps = mmps.tile([P, ROWS], dt.float32, tag="mm")
                    for kp in range(CCH // 2):
                        nc.tensor.matmul(
                            ps[:],
                            lhsT=wv8[:, 2 * kp:2 * kp + 2, mc * P:(mc + 1) * P],
                            rhs=HT8[:, 2 * kp:2 * kp + 2, :],
                            start=(kp == 0), stop=(kp == CCH // 2 - 1),
                            perf_mode=DR)
                    dst_col(mc, ps)

            def attn_layer(li):
                WQ = wpool.tile([P, CCH * C], dt.float8e4, tag="wq")
                WK = wpool.tile([P, CCH * C], dt.float8e4, tag="wk")
                WV = wpool.tile([P, CCH * C], dt.float8e4, tag="wv")
                WO = wpool.tile([P, CCH * C], dt.float8e4, tag="wo")
                nc.sync.dma_start(WK[:], wk_d.ap()[li])
                nc.sync.dma_start(WV[:], wv_d.ap()[li])
                nc.sync.dma_start(WQ[:], wq_d.ap()[li])
                nc.sync.dma_start(WO[:], wo_d.ap()[li])

                layer_norm(li)

                kin_ki, kout_ki = kin_k[li % 2], kout_k[li % 2]
                kin_vi, kout_vi = kin_v[li % 2], kout_v[li % 2]

                # K^T (own shard): bf16 for own scores + fp8 for transport
                def k_out(mc, ps):
                    nc.vector.tensor_scalar(
                        KSTG[:, mc * ROWS:(mc + 1) * ROWS], ps[:], IWS, None,
                        OP.mult)
                    nc.vector.tensor_scalar(
                        K8STG[:, mc * ROWS:(mc + 1) * ROWS], ps[:], IWS, None,
                        OP.mult)
                proj_dr(WK[:], k_out)
                nc.sync.dma_start(kin_ki.ap(), K8STG.bitcast(dt.bfloat16))
                if fake_ag:
                    for r in range(GROUP):
                        nc.sync.dma_start(kout_ki.ap()[r], kin_ki.ap())
                else:
                    nc.gpsimd.collective_compute(
                        "AllGather", mybir.AluOpType.bypass, replica_groups=groups,
                        ins=[kin_ki.ap().opt()], outs=[kout_ki.ap().opt()])

                # Q^T first: the own-shard score phase needs it, so it
                # starts earlier and covers more of the K AllGather flight
                proj_dr(WQ[:], lambda mc, ps: nc.vector.tensor_scalar(
                    QTSB[:, mc * ROWS:(mc + 1) * ROWS], ps[:], IWS, None, OP.mult))

                # V (own shard, natural, fp8, ones col)
                wv8 = WV[:].rearrange("p (cc c) -> p cc c", cc=CCH)
                for kt in range(KTO):
                    ps = mmps.tile([P, C], dt.float32, tag="mm")
                    for kp in range(CCH // 2):
                        nc.tensor.matmul(
                            ps[:],
                            lhsT=HT8[:, 2 * kp:2 * kp + 2, kt * P:(kt + 1) * P],
                            rhs=wv8[:, 2 * kp:2 * kp + 2, :],
                            start=(kp == 0), stop=(kp == CCH // 2 - 1),
                            perf_mode=DR)
                    nc.vector.tensor_scalar(
                        VSTG[:, kt, :, 0:CH],
                        ps[:].rearrange("p (h c) -> p h c", h=H), IWS, None, OP.mult)
                nc.sync.dma_start(
                    kin_vi.ap(),
                    VSTG.rearrange("p a b c -> p (a b c)").bitcast(dt.bfloat16))
                if fake_ag:
                    for r in range(GROUP):
                        nc.sync.dma_start(kout_vi.ap()[r], kin_vi.ap())
                else:
                    nc.gpsimd.collective_compute(
                        "AllGather", mybir.AluOpType.bypass, replica_groups=groups,
                        ins=[kin_vi.ap().opt()], outs=[kout_vi.ap().opt()])

                # unload the 3 REMOTE blocks (runtime-indexed by rank)
                if fake_ag:
                    srcs_k = [kout_ki.ap()[j] for j in range(1, GROUP)]
                    srcs_v = [kout_vi.ap()[j] for j in range(1, GROUP)]
                else:
                    srcs_k = [kout_ki.ap()[(me + j) % GROUP] for j in range(1, GROUP)]
                    srcs_v = [kout_vi.ap()[(me + j) % GROUP] for j in range(1, GROUP)]
                kt8v = KT8.rearrange("p (cc k) -> p cc k", cc=CCH)
                ktbv = KTSB.rearrange("p (cc k) -> p cc k", cc=CCH)
                for j in range(1, GROUP):
                    nc.sync.dma_start(
                        kt8v[:, :, (j - 1) * ROWS:j * ROWS].bitcast(dt.bfloat16),
                        srcs_k[j - 1].rearrange("p (cc r) -> p cc r", cc=CCH))
                    # upcast this block to bf16 for the score matmuls
                    nc.vector.tensor_copy(
                        ktbv[:, :, (j - 1) * ROWS:j * ROWS],
                        kt8v[:, :, (j - 1) * ROWS:j * ROWS])
                vsbf = VSB.rearrange("p a b c -> p (a b c)").bitcast(dt.bfloat16)
                for j in range(1, GROUP):
                    nc.sync.dma_start(vsbf[:, (j - 1) * VHALF:j * VHALF],
                                      srcs_v[j - 1])

                # ---- attention ----
                # esb slot s holds k-tile s: slots 0-3 own (KSTG/VSTG),
                # slots 4-15 remote (KTSB/VSB in arrival order).
                OG = [[0, 1, 2], [3]]                 # own k-slot groups
                RG = [[4, 5, 6], [7, 8, 9], [10, 11, 12], [13, 14, 15]]

                def score_mm(hp, sub, ps, j, slot):
                    if slot < KTO:
                        lt = KSTG[sub:sub + CH,
                                  hp * ROWS + slot * P: hp * ROWS + (slot + 1) * P]
                    else:
                        rk = slot - KTO
                        lt = KTSB[sub:sub + CH,
                                  hp * KTR * P + rk * P: hp * KTR * P + (rk + 1) * P]
                    nc.tensor.matmul(
                        ps[:, j * ROWS:(j + 1) * ROWS], lhsT=lt,
                        rhs=QTSB[sub:sub + CH, hp * ROWS:(hp + 1) * ROWS],
                        start=True, stop=True)

                def emit_groups(hp, esA, esB, glist, drain):
                    """staggered A/B score+exp emission over glist."""
                    for gi in range(len(glist) + 1):
                        todo = []
                        if gi >= 1:
                            psB = scps.tile([P, 3 * ROWS], dt.float32, tag="sc")
                            todo.append((CH, psB, esB, glist[gi - 1]))
                        if gi < len(glist):
                            psA = scps.tile([P, 3 * ROWS], dt.float32, tag="sc")
                            todo.append((0, psA, esA, glist[gi]))
                        for j in range(max(len(t[3]) for t in todo)):
                            for sub, ps, es, kts in todo:
                                if j < len(kts):
                                    score_mm(hp, sub, ps, j, kts[j])
                        for sub, ps, es, kts in todo:
                            nc.scalar.activation(
                                es[:, kts[0]:kts[0] + len(kts), :],
                                ps[:, 0:len(kts) * ROWS],
                                AF.Exp, scale=1.0 / np.sqrt(CH))
                        if drain:
                            for _ in range(4):
                                if drain:
                                    drain.pop(0)()

                def make_pair_work(hp, esA, esB):
                    work = []
                    avA = mmps.tile([P, ROWS], dt.float32, tag="mm")
                    avB = mmps.tile([P, ROWS], dt.float32, tag="mm")
                    for t in range(KT // 2):
                        for av, es in ((avA, esA), (avB, esB)):
                            h = 2 * hp + (0 if av is avA else 1)
                            if t < KTO // 2:
                                lt = VSTG[:, 2 * t:2 * t + 2, h, 0:80]
                            else:
                                rt_ = t - KTO // 2
                                lt = VSB[:, 2 * rt_:2 * rt_ + 2, h, 0:80]
                            work.append((lambda av=av, es=es, t=t, lt=lt: nc.tensor.matmul(
                                av[0:80, :], lhsT=lt,
                                rhs=es[:, 2 * t:2 * t + 2, :],
                                start=(t == 0), stop=(t == KT // 2 - 1),
                                perf_mode=DR)))

                    def post_head(j, av):
                        h = 2 * hp + j
                        sub = (h % 2) * CH
                        nc.vector.tensor_copy(
                            OSC[sub:sub + CH, hp, :], av[0:CH, :])
                        nc.vector.tensor_copy(
                            RECB[CH:CH + 1, j * ROWS:(j + 1) * ROWS],
                            av[CH:CH + 1, :])
                        bc = mmps.tile([P, ROWS], dt.float32, tag="mm")
                        nc.tensor.matmul(
                            bc[:, :], lhsT=ONES[CH:CH + 1, :],
                            rhs=RECB[CH:CH + 1, j * ROWS:(j + 1) * ROWS],
                            start=True, stop=True)
                        rb = small.tile([P, ROWS], dt.float32, tag="rb")
                        nc.vector.reciprocal_approx_fast(rb[:], bc[:, :])
                        dst = OSC[sub:sub + CH, hp, :]
                        nc.vector.tensor_mul(dst, dst, rb[sub:sub + CH, :])
                    work.append(lambda: post_head(0, avA))
                    work.append(lambda: post_head(1, avB))
                    return work

                # phase A: own k-tiles for all pairs (overlaps the AllGathers)
                esbs = []
                for hp in range(H // 2):
                    esA = epool.tile([P, KT, ROWS], dt.float8e4, tag="esA")
                    esB = epool.tile([P, KT, ROWS], dt.float8e4, tag="esB")
                    esbs.append((esA, esB))
                    emit_groups(hp, esA, esB, OG, None)
                # phase B: remote k-tiles; AV of the previous pair interleaves
                pending = []
                for hp in range(H // 2):
                    esA, esB = esbs[hp]
                    emit_groups(hp, esA, esB, RG, pending)
                    while pending:
                        pending.pop(0)()
                    pending = make_pair_work(hp, esA, esB)
                while pending:
                    pending.pop(0)()

                # out-projection (fp8 DR) + residual
                if use_bo:
                    BO = gpool.tile([P, C], dt.float32, tag="bo")
                    nc.sync.dma_start(BO[:], bo_d.ap()[li])
                wo8 = WO[:].rearrange("p (cc c) -> p cc c", cc=CCH)
                for rt in range(RT):
                    ps = mmps.tile([P, C], dt.float32, tag="mm")
                    for kp in range(CCH // 2):
                        nc.tensor.matmul(
                            ps[:],
                            lhsT=OSC[:, 2 * kp:2 * kp + 2, rt * P:(rt + 1) * P],
                            rhs=wo8[:, 2 * kp:2 * kp + 2, :],
                            start=(kp == 0), stop=(kp == CCH // 2 - 1),
                            perf_mode=DR)
                    dst = XSB[:, rt * C:(rt + 1) * C]
                    nc.vector.scalar_tensor_tensor(dst, ps[:], IWS, dst,
                                                   OP.mult, OP.add)
                    if use_bo:
                        nc.vector.tensor_add(dst, dst, BO[:])

            for li in range(n_layers):
                attn_layer(li)

            # ---- FFN (fp8 DR) ----
            layer_norm(L)
            WG = wpool.tile([P, CCH * C], dt.float8e4, tag="wq")
            WF = wpool.tile([P, 2 * C], dt.float8e4, tag="wf")
            nc.sync.dma_start(WG[:], wg_d.ap())
            nc.sync.dma_start(WF[:], wf_d.ap())
            if use_bg:
                BG = gpool.tile([P, CCH], dt.float32, tag="bg")
                nc.sync.dma_start(BG[:], bg_d.ap())
            AGT = small.tile([P, 2 * ROWS], dt.bfloat16, tag="ffa")
            GGT = small.tile([P, 2 * ROWS], dt.bfloat16, tag="ffg")
            wg8 = WG[:].rearrange("p (cc c) -> p cc c", cc=CCH)
            for mg in range(CCH):
                ps = mmps.tile([P, ROWS], dt.float32, tag="mm")
                for kp in range(CCH // 2):
                    nc.tensor.matmul(
                        ps[:],
                        lhsT=wg8[:, 2 * kp:2 * kp + 2, mg * P:(mg + 1) * P],
                        rhs=HT8[:, 2 * kp:2 * kp + 2, :],
                        start=(kp == 0), stop=(kp == CCH // 2 - 1),
                        perf_mode=DR)
                dst = (AGT if mg < 2 else GGT)[:, (mg % 2) * ROWS:(mg % 2 + 1) * ROWS]
                if use_bg:
                    nc.vector.tensor_scalar(dst, ps[:], IWS, BG[:, mg:mg + 1],
                                            OP.mult, OP.add)
                else:
                    nc.vector.tensor_scalar(dst, ps[:], IWS, None, OP.mult)
            K1 = 0.7978845608
            for j in range(2):
                ga = GGT[:, j * ROWS:(j + 1) * ROWS]
                aa = AGT[:, j * ROWS:(j + 1) * ROWS]
                SQ = small.tile([P, ROWS], dt.float32, tag="sq")
                WT = small.tile([P, ROWS], dt.float32, tag="wt")
                VV = small.tile([P, ROWS], dt.float32, tag="vv")
                TT = small.tile([P, ROWS], dt.float32, tag="tt")
                HT2 = small.tile([P, ROWS], dt.bfloat16, tag="ht2")
                PP = small.tile([P, ROWS], dt.bfloat16, tag="pp")
                nc.scalar.activation(SQ[:], ga, AF.Square)
                nc.vector.tensor_scalar(WT[:], SQ[:], K1 * 0.044715, K1,
                                        OP.mult, OP.add)
                nc.vector.tensor_mul(VV[:], ga, WT[:])
                nc.scalar.activation(TT[:], VV[:], AF.Tanh)
                nc.vector.tensor_scalar(HT2[:], TT[:], 0.5, 0.5,
                                        OP.mult, OP.add)
                nc.vector.tensor_mul(PP[:], aa, ga)
                nc.vector.tensor_mul(FFSB[:, j, :], PP[:], HT2[:])
            if use_bf:
                BF = gpool.tile([P, C], dt.float32, tag="bf")
                nc.sync.dma_start(BF[:], bf_d.ap())
            wf8 = WF[:].rearrange("p (two c) -> p two c", two=2)
            for rt in range(RT):
                ps = mmps.tile([P, C], dt.float32, tag="mm")
                nc.tensor.matmul(
                    ps[:],
                    lhsT=FFSB[:, 0:2, rt * P:(rt + 1) * P],
                    rhs=wf8[:, 0:2, :],
                    start=True, stop=True, perf_mode=DR)
                OUT = small.tile([P, C], dt.float32, tag="out")
                nc.vector.scalar_tensor_tensor(OUT[:], ps[:], IWS,
                                               XSB[:, rt * C:(rt + 1) * C],
                                               OP.mult, OP.add)
                if use_bf:
                    nc.vector.tensor_add(OUT[:], OUT[:], BF[:])
                nc.sync.dma_start(y_d.ap()[rt * P:(rt + 1) * P, :], OUT[:])

    nc.compile()
    return nc


def kernel(x, ln_gamma, ln_beta, Wq, Wk, Wv, Wo, bo, Wg, bg, Wf, bf):
    x = np.asarray(x, np.float32)
    ln_gamma = np.asarray(ln_gamma, np.float32)
    ln_beta = np.asarray(ln_beta, np.float32)
    Wq, Wk, Wv, Wo = (np.asarray(w, np.float32) for w in (Wq, Wk, Wv, Wo))
    bo = np.asarray(bo, np.float32)
    Wg, Wf = np.asarray(Wg, np.float32), np.asarray(Wf, np.float32)
    bg, bf = np.asarray(bg, np.float32), np.asarray(bf, np.float32)

    use_gamma = not np.all(ln_gamma == 1.0)
    use_beta = not np.all(ln_beta == 0.0)
    use_bo = not np.all(bo == 0.0)
    use_bg = not np.all(bg == 0.0)
    use_bf = not np.all(bf == 0.0)
    flags = (use_gamma, use_beta, use_bo, use_bg, use_bf)

    if flags not in _CACHE:
        _CACHE[flags] = _build(flags)
    nc = _CACHE[flags]

    ws = WSCALE
    wq_h = np.stack([_swz(Wq[l] * ws, CCH) for l in range(L)]).astype(F8)
    wk_h = np.stack([_swz(Wk[l] * ws, CCH) for l in range(L)]).astype(F8)
    wv_h = np.stack([_swz(Wv[l] * ws, CCH) for l in range(L)]).astype(F8)
    wo_h = np.stack([_swz(Wo[l] * ws, CCH) for l in range(L)]).astype(F8)
    wg_h = _swz(Wg * ws, CCH).astype(F8)
    wf_h = _swz(Wf * ws, 2).astype(F8)

    xf = x.reshape(B * S, C)
    base = {
        "wq": wq_h, "wk": wk_h, "wv": wv_h, "wo": wo_h,
        "wg": wg_h, "wf": wf_h,
    }
    if use_gamma:
        base["gam"] = np.ascontiguousarray(
            np.broadcast_to(ln_gamma[:, None, :], (L + 1, P, C))).astype(BF16)
    if use_beta:
        base["bet"] = np.ascontiguousarray(
            np.broadcast_to(ln_beta[:, None, :], (L + 1, P, C))).astype(BF16)
    if use_bo:
        base["bob"] = np.ascontiguousarray(
            np.broadcast_to(bo[:, None, :], (L, P, C))).astype(np.float32)
    if use_bg:
        base["bgc"] = np.ascontiguousarray(bg.reshape(CCH, P).T).astype(np.float32)
    if use_bf:
        base["bfb"] = np.ascontiguousarray(
            np.broadcast_to(bf[None, :], (P, C))).astype(np.float32)

    in_maps = []
    for c in range(N_CORES):
        m = dict(base)
        m["x"] = np.ascontiguousarray(xf[c * ROWS:(c + 1) * ROWS])
        in_maps.append(m)

    from concourse.bass_utils import run_bass_kernel_spmd
    res = run_bass_kernel_spmd(nc, in_maps, core_ids=list(range(N_CORES)))
    out = np.concatenate([res.results[c]["y"] for c in range(N_CORES)], axis=0)
    return out.reshape(B, S, C).astype(np.float32)
